# revision 68
# baseline (speedup 1.0000x reference)
"""Trainium2 Bass kernel for nn_CompressedInteractionNetwork_9105330667837.

Algorithm: the network output is (B,1) only, so the 3-layer CIN collapses
algebraically to a per-(b,d)-column quartic form evaluated as
    out[b] = B_const + sum_d [ g(x).t(x) + x.u(x) ],   x = x0[b,:,d] in R^32
with g[o] = x^T W1[o] x (64 quadratic forms), t[k] = x^T U3[k] x + V2[k].x,
u = Asym x + s23.  All quadratic forms are evaluated through a shared
"squares basis": z = LIN @ x (pair-sums), basis = [z^2; x^2; x_m x_{m+16}; x],
then [g;t] = R @ basis.  Everything contracts on the PE in float32r; squares
on ScalarE; products/reduction on VectorE/GpSimd.

Sharding: data-parallel over batch across 8 cores (weights replicated).
"""

import numpy as np
from contextlib import ExitStack

import concourse.bass as bass
from concourse import bacc
import concourse.mybir as mybir
import concourse.tile as tile
from concourse.bass_utils import run_bass_kernel_spmd
from concourse import dve_ops as _dvo
from concourse.dve_spec import Spec as _Spec, Src0 as _Src0, Bin as _Bin, AluOp as _AluOp
from concourse.dve_table_gen import dve_ver_for as _dve_ver_for


def _register_square_op():
    if "SQUARE_ANT" in _dvo._SUB_OPCODE_FOR_NAME:
        return _dvo.CUSTOM_DVE_SPECS and [op for op in _dvo.OPS if op.name == "SQUARE_ANT"][0]
    op = _dvo.DveOp(
        "SQUARE_ANT",
        _Spec(
            body=_Bin(_AluOp.MULTIPLY, _Src0, _Src0),
            reference=lambda in0, in1, s0, s1, imm2: (
                in0.astype(np.float32) * in0.astype(np.float32)
            ),
        ),
        subdim=False,
        uops_sha={},
    )
    _dvo.OPS.append(op)
    _dvo.CUSTOM_DVE_SPECS[op.name] = op.spec
    _dvo._SUB_OPCODE_FOR_NAME[op.name] = max(_dvo._SUB_OPCODE_FOR_NAME.values()) + 1
    for ver in ("v3", "v4"):
        try:
            op.compile(ver)
        except ValueError as e:
            import re as _re
            m = _re.search(r": ([0-9a-f]{16}) ", str(e))
            if m is None:
                raise
            op.uops_sha[ver] = m.group(1)
            _dvo._COMPILE_CACHE.pop((op.name, ver), None)
            op.compile(ver)
    return op


SQUARE_ANT = _register_square_op()


def _patch_dve_root_path():
    """walrus requires --dve-root-json absolute; the bass2jax hook path can
    hand it a relative path. Resolve against plausible bases."""
    import os
    import concourse.bass_utils as bu

    if getattr(bu, "_dve_path_patched", False):
        return
    orig = bu.run_command

    def patched(cmd, cwd=None, **kw):
        try:
            i = cmd.index("--dve-root-json") + 1
            p = cmd[i]
            if not os.path.isabs(p):
                for base in (cwd or ".", os.path.join(cwd or ".", ".."), "."):
                    cand = os.path.abspath(os.path.join(base, p))
                    if os.path.exists(cand):
                        cmd[i] = cand
                        break
        except ValueError:
            pass
        return orig(cmd, cwd=cwd, **kw)

    bu.run_command = patched
    bu._dve_path_patched = True


_patch_dve_root_path()

B, F, D = 2048, 32, 64
NCORES = 8
BC = B // NCORES            # 256 batches per core
CHUNK_B = 8                 # batches per chunk
P = CHUNK_B * D             # 512 pairs per chunk
NCHUNK = BC // CHUNK_B      # 32

SPECIAL = [(m, m + 16) for m in range(16)]          # pairs done as direct products
_SP = set(SPECIAL)
PAIRS = [(a, b) for a in range(F) for b in range(a + 1, F) if (a, b) not in _SP]
assert len(PAIRS) == 480

f32 = mybir.dt.float32
f32r = mybir.dt.float32r


def fold_weights(W1, b1, W2, b2, W3, b3, W_out, b_out):
    """Host-side folding. Returns dict of small fp32 arrays + bconst float."""
    W1, b1, W2, b2, W3, b3, W_out, b_out = [
        np.asarray(a, dtype=np.float64) for a in (W1, b1, W2, b2, W3, b3, W_out, b_out)
    ]
    w1, w2, w3 = W_out[0:64, 0], W_out[64:128, 0], W_out[128:192, 0]

    V2 = np.einsum("o,ohm->hm", w2, W2)           # (64,32)
    V3 = np.einsum("o,ohm->hm", w3, W3)           # (64,32)
    U3 = np.einsum("hkm,hn->kmn", W2, V3)         # (64,32,32)
    V1 = np.einsum("o,ohm->hm", w1, W1)           # (32,32)
    Le = np.einsum("k,kmn->mn", b1, U3)           # (32,32)
    A = V1 + Le
    Asym = (A + A.T) / 2
    s23 = V2.T @ b1 + V3.T @ b2                   # (32,)
    bconst = D * (w1 @ b1 + w2 @ b2 + w3 @ b3) + b_out[0]

    M1s = (W1 + W1.transpose(0, 2, 1)) / 2        # 64 sym forms for g
    U3s = (U3 + U3.transpose(0, 2, 1)) / 2        # 64 sym forms for t

    # LIN lhsT: (32, 4*128). Tile j rows: j<3 -> PAIRS[128j:128j+128] sums;
    # tile 3 -> PAIRS[384:480] sums (96 rows) + Asym rows (32).
    LINW = np.zeros((F, 4 * 128))
    for j in range(4):
        rows = PAIRS[128 * j: 128 * (j + 1)]
        for i, (a, b_) in enumerate(rows):
            LINW[a, 128 * j + i] += 1.0
            LINW[b_, 128 * j + i] += 1.0
        if j == 3:
            LINW[:, 128 * 3 + 96: 128 * 3 + 128] = Asym.T  # rows 96..127 = Asym @ x

    # Big-matmul lhsT per chain: RW (128, 5*128): RW[k, 128j+? ...] wait layout:
    # lhsT for chain j is (K_j, 128): RW[0:K_j, j-block], K_j = 128 (j<4) or 48.
    # outputs: m<64 -> form M1s[m], v=0 ; m>=64 -> form U3s[m-64], v=V2[m-64]
    forms = np.concatenate([M1s, U3s], axis=0)    # (128, 32, 32)
    linv = np.concatenate([np.zeros((64, F)), V2], axis=0)  # (128, 32)

    # rw layout: rw[k, 128*j + m] = weight of chain-j basis-row k for output m
    RW = np.zeros((128, 5 * 128))
    # chains 0-2: squares of pair-sums
    for j in range(3):
        rows = PAIRS[128 * j: 128 * (j + 1)]
        for i, (a, b_) in enumerate(rows):
            RW[i, 128 * j:128 * (j + 1)] = forms[:, a, b_]
    # chain 3: rows 0-95 squares of PAIRS[384:480]; rows 96-127 x^2
    for i, (a, b_) in enumerate(PAIRS[384:480]):
        RW[i, 128 * 3:128 * 4] = forms[:, a, b_]
    # x^2 weights: S[m,m] - sum_{(a,b) in PAIRS containing m} S[a,b]
    corr = np.zeros((128, F))
    for (a, b_) in PAIRS:
        corr[:, a] += forms[:, a, b_]
        corr[:, b_] += forms[:, a, b_]
    # chain 4 (K=80): rows 0-31 x^2; 32-63 x; 64-79 products x_m x_{m+16}
    for i, (a, b_) in enumerate(SPECIAL):
        RW[64 + i, 128 * 4:128 * 5] = 2.0 * forms[:, a, b_]
    for m in range(F):
        RW[32 + m, 128 * 4:128 * 5] = linv[:, m]
        RW[m, 128 * 4:128 * 5] = forms[:, m, m] - corr[:, m]

    return {
        "linw": LINW.astype(np.float32),
        "rw": RW.astype(np.float32),
        "s23": s23.reshape(F, 1).astype(np.float32),
        "ones": np.ones((96, 1), dtype=np.float32),
    }, float(bconst)


PAIRS2 = [(a, b) for a in range(F) for b in range(a + 1, F)]   # all 496
assert len(PAIRS2) == 496
NV = 577            # v' = [z2(512, 16 spare); x2(32); x(32); const(1)]
IX_X2 = 512
IX_X = 544
IX_C = 576


def _form_to_v(S):
    """Coefficient vector c in R^577 with c.v' = x^T S x (S sym, 32x32)."""
    c = np.zeros(NV)
    for i, (a, b) in enumerate(PAIRS2):
        j, k = divmod(i, 128)
        c[128 * j + k] = S[a, b]
    for m in range(F):
        c[IX_X2 + m] = S[m, m] - sum(
            S[a, b] for (a, b) in PAIRS2 if a == m or b == m
        )
    return c


def fold_weights2(W1, b1, W2, b2, W3, b3, W_out, b_out):
    """v2 folding: quartic = sum_i s_i * (rw2 @ v')_i^2 + bconst, where
    v' = [pair-sum squares(496+16 spare); x^2(32); x(32); 1]."""
    W1, b1, W2, b2, W3, b3, W_out, b_out = [
        np.asarray(a, dtype=np.float64) for a in (W1, b1, W2, b2, W3, b3, W_out, b_out)
    ]
    w1, w2, w3 = W_out[0:64, 0], W_out[64:128, 0], W_out[128:192, 0]
    V2 = np.einsum("o,ohm->hm", w2, W2)           # (64,32)
    V3 = np.einsum("o,ohm->hm", w3, W3)           # (64,32)
    U3 = np.einsum("hkm,hn->kmn", W2, V3)         # (64,32,32)

    M1s = (W1 + W1.transpose(0, 2, 1)) / 2
    U3s = (U3 + U3.transpose(0, 2, 1)) / 2

    # Bias-folded pairing: total = sum_k ghat_k that_k + (V3^T b2).x + bconst
    # with ghat_k = x^T M1s[k] x + b1_k, that_k = x^T U3s[k] x + V2[k].x + w1_k
    # and bconst = D*(w2.b2 + w3.b3) + b_out.
    G = np.stack([_form_to_v(M1s[k]) for k in range(64)])      # (64, 577)
    T = np.stack([_form_to_v(U3s[k]) for k in range(64)])      # (64, 577)
    for k in range(64):
        G[k, IX_C] = b1[k]
        T[k, IX_X:IX_X + F] += V2[k]
        T[k, IX_C] = w1[k]
    ell = np.zeros(NV)
    ell[IX_X:IX_X + F] = V3.T @ b2
    bconst = D * (w2 @ b2 + w3 @ b3) + b_out[0]

    C = G.T @ T
    C[IX_C, :] += ell
    Cs = (C + C.T) / 2
    lam, vec = np.linalg.eigh(Cs)
    order = np.argsort(-np.abs(lam))
    keep = order[:128]
    trunc = float(np.abs(lam[order[128:]]).sum())
    lead = float(np.abs(lam[keep]).sum())
    rw2 = (vec[:, keep] * np.sqrt(np.abs(lam[keep]))).T        # (128, 577)
    signs = np.sign(lam[keep])

    # LIN lhsT: (32*4 partitions, 4*128 cols): chain j rows 32j..32j+32,
    # cols 128j..128(j+1); pair-sum weights.
    LIN2 = np.zeros((128, 4 * 128))
    for i, (a, b) in enumerate(PAIRS2):
        j, k = divmod(i, 128)
        LIN2[32 * j + a, 128 * j + k] = 1.0
        LIN2[32 * j + b, 128 * j + k] = 1.0

    # big-matmul lhsT blocks: chains j<4: (128,128) = rw2 cols 128j..;
    # chain 4: (65, 128) = rw2 cols [x2, x, const]
    RWB = np.zeros((128, 5 * 128))
    for j in range(4):
        RWB[:, 128 * j:128 * (j + 1)] = rw2[:, 128 * j:128 * (j + 1)].T
    RWB[0:65, 512:640] = rw2[:, 512:577].T

    return {
        "linw2": LIN2.astype(np.float32),
        "rwb": RWB.astype(np.float32),
        "signs": signs.reshape(128, 1).astype(np.float32),
    }, float(bconst), {"trunc": trunc, "lead": lead}


_module_cache = {}


CFG = {"P": 512, "lin_split": 2, "lin_bufs": 2, "big_bufs": 2, "gp_d16": True,
       "dve_sq": False,
       "no_sq": False, "no_gs": False, "no_dve": False, "no_gp": False,
       "no_lin": False, "no_big": False, "no_dma": False}


def build_module(bconst: float, repeat: int = 1):
    key = (round(bconst, 12), repeat, tuple(sorted(CFG.items())))
    if key in _module_cache:
        return _module_cache[key]
    nc = bacc.Bacc("TRN2", target_bir_lowering=False)
    x_d = nc.dram_tensor("x", [BC, F, D], f32r, kind="ExternalInput")
    linw_d = nc.dram_tensor("linw", [F, 4 * 128], f32r, kind="ExternalInput")
    rw_d = nc.dram_tensor("rw", [128, 5 * 128], f32r, kind="ExternalInput")
    s23_d = nc.dram_tensor("s23", [F, 1], f32, kind="ExternalInput")
    ones_d = nc.dram_tensor("ones", [96, 1], f32r, kind="ExternalInput")
    out_d = nc.dram_tensor("out", [1, BC], f32, kind="ExternalOutput")

    SQ = mybir.ActivationFunctionType.Square
    CP = mybir.ActivationFunctionType.Copy
    ADD = mybir.AluOpType.add
    MULT = mybir.AluOpType.mult

    with tile.TileContext(nc) as tc, ExitStack() as ctx:
        const = ctx.enter_context(tc.tile_pool(name="const", bufs=1))
        xp = ctx.enter_context(tc.tile_pool(name="xp", bufs=3))
        chp = ctx.enter_context(tc.tile_pool(name="chp", bufs=10))
        ch4p = ctx.enter_context(tc.tile_pool(name="ch4p", bufs=3))
        prp = ctx.enter_context(tc.tile_pool(name="prp", bufs=1))
        gsp = ctx.enter_context(tc.tile_pool(name="gsp", bufs=3))
        outp = ctx.enter_context(tc.tile_pool(name="outp", bufs=1))
        linps = ctx.enter_context(
            tc.tile_pool(name="linps", bufs=CFG["lin_bufs"], space="PSUM"))
        bigps = ctx.enter_context(tc.tile_pool(name="bigps", bufs=CFG["big_bufs"], space="PSUM"))
        finps = ctx.enter_context(tc.tile_pool(name="finps", bufs=1, space="PSUM"))

        linw_t = const.tile([F, 4 * 128], f32r)
        nc.sync.dma_start(linw_t[:], linw_d[:])
        rw_t = const.tile([128, 5 * 128], f32r)
        nc.sync.dma_start(rw_t[:], rw_d[:])
        s23_t = const.tile([F, 1], f32)
        nc.sync.dma_start(s23_t[:], s23_d[:])
        ones_t = const.tile([96, 1], f32r)
        nc.sync.dma_start(ones_t[:], ones_d[:])

        pr_all = prp.tile([96, BC * D], f32r)

        rep_ctx = tc.For_i(0, repeat, 1) if repeat > 1 else None
        if rep_ctx is not None:
            ctx.enter_context(rep_ctx)

        CP_ = CFG["P"]          # pairs per chunk
        CB = CP_ // D           # batches per chunk
        NCH = BC // CB          # chunks
        NT = CP_ // 512         # matmul N-tiles per chunk
        GROUP = max(1, 2048 // CP_)
        for g in range(NCH // GROUP):
            b0 = g * GROUP * CB
            nb = GROUP * CB
            xsrc = x_d[b0:b0 + nb].transpose([1, 0, 2])   # (32, nb, 64)
            xg_t = xp.tile([F, GROUP * CP_], f32r, tag="x")
            nc.sync.dma_start(
                xg_t[:].rearrange("k (b d) -> k b d", b=nb), xsrc
            )
            ch4g = ch4p.tile([80, GROUP * CP_], f32r, tag="ch4")
            nc.sync.dma_start(
                ch4g[32:64].rearrange("k (b d) -> k b d", b=nb), xsrc
            )
            xs_g = xp.tile([16, GROUP * CP_], f32r, tag="xs")
            nc.sync.dma_start(
                xs_g[:].rearrange("k (b d) -> k b d", b=nb),
                x_d[b0:b0 + nb, 16:32, :].transpose([1, 0, 2]),
            )
            for ci in range(GROUP):
                cs = slice(ci * CP_, (ci + 1) * CP_)
                x_t = xg_t[:, cs]
                ch4 = ch4g[:, cs]

                ns = CFG["lin_split"]  # chains per lin psum tile
                ntile = 4 // ns
                lptiles = []
                chains = []
                for t in range(ntile):
                    lp = linps.tile([128, ns * CP_], f32, tag="lp")
                    for jj in range(ns):
                        j = t * ns + jj
                        for nt in range(NT):
                            nc.tensor.matmul(
                                lp[:, jj * CP_ + nt * 512:jj * CP_ + (nt + 1) * 512],
                                linw_t[:, 128 * j:128 * (j + 1)],
                                x_t[:, nt * 512:(nt + 1) * 512],
                                start=True, stop=True,
                            )
                    lptiles.append(lp)
                    chn = chp.tile([128, ns * CP_], f32r, tag="ch")
                    c_ = g * GROUP + ci
                    if t == ntile - 1 and ntile > 1 and c_ % 2 == 1 and CFG["dve_sq"]:
                        nc.vector._custom_dve(SQUARE_ANT, out=chn[:], in0=lp[:])
                    else:
                        nc.scalar.activation(chn[:], lp[:], SQ)
                    for jj in range(ns):
                        chains.append(chn[:, jj * CP_:(jj + 1) * CP_])
                lp_u = lptiles[-1]
                nc.gpsimd.tensor_mul(ch4[0:32], x_t, x_t)
                if CFG["gp_d16"]:
                    nc.gpsimd.tensor_mul(ch4[64:80], x_t[0:16], xs_g[:, cs])
                else:
                    nc.vector.tensor_mul(ch4[64:80], x_t[0:16], xs_g[:, cs])
                chains.append(ch4)

                bp = bigps.tile([128, CP_], f32, tag="bp")
                for j in range(5):
                    K_j = 128 if j < 4 else 80
                    for nt in range(NT):
                        nc.tensor.matmul(
                            bp[:, nt * 512:(nt + 1) * 512],
                            rw_t[0:K_j, 128 * j:128 * (j + 1)],
                            chains[j][0:K_j, nt * 512:(nt + 1) * 512],
                            start=(j == 0), stop=(j == 4),
                        )

                c = g * GROUP + ci
                pcs = slice(c * CP_, (c + 1) * CP_)
                gs = gsp.tile([64, CP_], f32, tag="gs")
                nc.scalar.activation(gs[:], bp[0:64], CP)
                nc.vector.tensor_mul(pr_all[0:64, pcs], gs[:], bp[64:128])
                nc.vector.scalar_tensor_tensor(
                    pr_all[64:96, pcs], lp_u[96:128, (ns - 1) * CP_:ns * CP_],
                    s23_t[:], x_t, ADD, MULT
                )

        fp = finps.tile([1, BC], f32)
        pr3 = pr_all[:].rearrange("p (b d) -> p b d", b=BC)
        for d in range(D):
            nc.tensor.matmul(
                fp[:], ones_t[:], pr3[:, :, d],
                start=(d == 0), stop=(d == D - 1),
            )
        out_sb = outp.tile([1, BC], f32)
        nc.scalar.activation(out_sb[:], fp[:], CP, bias=float(bconst))
        nc.sync.dma_start(out_d[:], out_sb[:])

    nc.compile()
    _module_cache[key] = nc
    return nc


bf16 = mybir.dt.bfloat16

# v2 config: engine assignment for the 4 z^2 ops, x^2 op, y^2 op.
# engines: "s" = scalar (activation Square), "v" = vector (custom dve square),
# "g" = gpsimd (tensor_mul; SBUF operands only!)
CFG2 = {
    "sq_eng": ["s", "v", "s", "v"],   # per LIN chain z^2
    "x2_eng": "g",
    "y2_eng": "s",
    "group": 4,
}


def build_module2(bconst: float, repeat: int = 1):
    key = ("v2", round(bconst, 12), repeat,
           tuple(CFG2["sq_eng"]), CFG2["x2_eng"], CFG2["y2_eng"], CFG2["group"])
    if key in _module_cache:
        return _module_cache[key]
    nc = bacc.Bacc("TRN2", target_bir_lowering=False)
    x_d = nc.dram_tensor("x", [BC, F, D], f32r, kind="ExternalInput")
    linw_d = nc.dram_tensor("linw2", [128, 4 * 128], f32r, kind="ExternalInput")
    rwb_d = nc.dram_tensor("rwb", [128, 5 * 128], f32r, kind="ExternalInput")
    signs_d = nc.dram_tensor("signs", [128, 1], bf16, kind="ExternalInput")
    ones_d = nc.dram_tensor("ones2", [1, CFG2["group"] * 512], f32r,
                            kind="ExternalInput")
    out_d = nc.dram_tensor("out", [1, BC], f32, kind="ExternalOutput")

    SQ = mybir.ActivationFunctionType.Square
    CP_ = 512                  # pair-cols per chunk
    CB = CP_ // D              # 8 batches per chunk
    NCH = BC // CB             # 32 chunks
    GROUP = CFG2["group"]      # chunks per DMA group
    QCH = 8                    # chunks per tail quarter (64 batches)

    def sq_op(eng, out, in_):
        if eng == "s":
            nc.scalar.activation(out, in_, SQ)
        elif eng == "v":
            nc.vector._custom_dve(SQUARE_ANT, out=out, in0=in_)
        else:
            nc.gpsimd.tensor_mul(out, in_, in_)

    with tile.TileContext(nc) as tc, ExitStack() as ctx:
        const = ctx.enter_context(tc.tile_pool(name="const", bufs=1))
        x4p = ctx.enter_context(tc.tile_pool(name="x4p", bufs=2))
        ch4p = ctx.enter_context(tc.tile_pool(name="ch4p", bufs=2))
        chp = ctx.enter_context(tc.tile_pool(name="chp", bufs=4))
        prp = ctx.enter_context(tc.tile_pool(name="prp", bufs=2))
        outp = ctx.enter_context(tc.tile_pool(name="outp", bufs=1))
        zpsA = ctx.enter_context(tc.tile_pool(name="zpsA", bufs=1, space="PSUM"))
        zpsB = ctx.enter_context(tc.tile_pool(name="zpsB", bufs=1, space="PSUM"))
        yps = ctx.enter_context(tc.tile_pool(name="yps", bufs=2, space="PSUM"))
        qps = ctx.enter_context(tc.tile_pool(name="qps", bufs=2, space="PSUM"))

        linw_t = const.tile([128, 4 * 128], f32r)
        nc.sync.dma_start(linw_t[:], linw_d[:])
        rwb_t = const.tile([128, 5 * 128], f32r)
        nc.sync.dma_start(rwb_t[:], rwb_d[:])
        signs_t = const.tile([128, 1], bf16)
        nc.sync.dma_start(signs_t[:], signs_d[:])
        out_acc = outp.tile([1, BC], f32)
        out_fin = outp.tile([1, BC], f32)

        rep_ctx = tc.For_i(0, repeat, 1) if repeat > 1 else None
        if rep_ctx is not None:
            ctx.enter_context(rep_ctx)

        def stage_a(x4_t, ch4g, cs):
            zA = zpsA.tile([128, 2 * CP_], f32, tag="zA")
            zB = zpsB.tile([128, 2 * CP_], f32, tag="zB")
            for j, (zt, off) in enumerate(
                    [(zA, 0), (zA, CP_), (zB, 0), (zB, CP_)]):
                nc.tensor.matmul(
                    zt[:, off:off + CP_],
                    linw_t[32 * j:32 * (j + 1), 128 * j:128 * (j + 1)],
                    x4_t[32 * j:32 * (j + 1), cs],
                    start=True, stop=True,
                    tile_position=(32 * j, 0),
                )
            chA = chp.tile([128, 2 * CP_], f32r, tag="chA")
            chB = chp.tile([128, 2 * CP_], f32r, tag="chB")
            nc.scalar.activation(chA[:], zA[:], SQ)
            nc.vector._custom_dve(SQUARE_ANT, out=chB[:], in0=zB[:])
            nc.gpsimd.tensor_mul(ch4g[0:32, cs], x4_t[0:32, cs], x4_t[0:32, cs])
            return (chA, chB, ch4g, cs)

        def stage_b(st, pr_t, pc):
            chA, chB, ch4g, cs = st
            yt = yps.tile([128, CP_], f32, tag="y")
            rhs = [chA[:, 0:CP_], chA[:, CP_:2 * CP_],
                   chB[:, 0:CP_], chB[:, CP_:2 * CP_]]
            for j in range(4):
                nc.tensor.matmul(yt[:], rwb_t[:, 128 * j:128 * (j + 1)],
                                 rhs[j], start=(j == 0), stop=False)
            nc.tensor.matmul(yt[:], rwb_t[0:65, 512:640], ch4g[0:65, cs],
                             start=False, stop=True)
            h = CP_ // 2
            nc.scalar.activation(pr_t[:, pc:pc + h], yt[:, 0:h], SQ)
            nc.vector._custom_dve(SQUARE_ANT, out=pr_t[:, pc + h:pc + CP_],
                                  in0=yt[:, h:CP_])

        NQ = NCH // QCH
        for q in range(NQ):                    # tail quarters (64 batches)
            pr_t = prp.tile([128, QCH * CP_], bf16, tag="pr")
            pend = None
            for gg in range(QCH // GROUP):     # DMA groups within quarter
                g = q * (QCH // GROUP) + gg
                b0 = g * GROUP * CB
                nb = GROUP * CB
                xsrc = x_d[b0:b0 + nb].transpose([1, 0, 2])   # (32, nb, 64)
                x4_t = x4p.tile([128, GROUP * CP_], f32r, tag="x4")
                for j in range(4):
                    nc.sync.dma_start(
                        x4_t[32 * j:32 * (j + 1)].rearrange(
                            "k (b d) -> k b d", b=nb), xsrc)
                ch4g = ch4p.tile([65, GROUP * CP_], f32r, tag="ch4")
                nc.sync.dma_start(
                    ch4g[32:64].rearrange("k (b d) -> k b d", b=nb), xsrc)
                if g < 2:
                    nc.sync.dma_start(ch4g[64:65], ones_d[:])

                for ci in range(GROUP):
                    cs = slice(ci * CP_, (ci + 1) * CP_)
                    c_in_q = gg * GROUP + ci
                    st = stage_a(x4_t, ch4g, cs)
                    if pend is not None:
                        stage_b(*pend)
                    pend = (st, pr_t, c_in_q * CP_)
            stage_b(*pend)
            # tail: 8 accumulating N=512 passes (8-d blocks), then a
            # segmented 8->1 reduce on VectorE into out_acc.
            qacc = qps.tile([1, CP_], f32, tag="qacc")
            pr4 = pr_t[:].rearrange("p (b d) -> p b d", d=D)
            for dd in range(8):
                nc.tensor.matmul(
                    qacc[:], signs_t[:], pr4[:, :, 8 * dd:8 * (dd + 1)],
                    start=(dd == 0), stop=(dd == 7),
                )
            nc.vector.tensor_reduce(
                out_acc[0:1, q * QCH * CB:(q + 1) * QCH * CB],
                qacc[:].rearrange("p (b d) -> p b d", d=8),
                mybir.AxisListType.X, mybir.AluOpType.add,
            )
        nc.scalar.activation(out_fin[:], out_acc[:],
                             mybir.ActivationFunctionType.Copy,
                             bias=float(bconst))
        nc.sync.dma_start(out_d[:], out_fin[:])

    nc.compile()
    _module_cache[key] = nc
    return nc


def build_module3(bconst: float, repeat: int = 1):
    """v3: same math as v2, restructured for back-to-back PE issue.

    - 4 LIN chains write 4 separate single-bank PSUM tiles (pool depth 5)
      so the next chunk's LIN matmuls never wait on this chunk's squares.
    - squares: 2 on ScalarE + 2 on VectorE per chunk; yt square split as in v2.
    """
    key = ("v3", round(bconst, 12), repeat)
    if key in _module_cache:
        return _module_cache[key]
    nc = bacc.Bacc("TRN2", target_bir_lowering=False)
    x_d = nc.dram_tensor("x", [BC, F, D], f32r, kind="ExternalInput")
    linw_d = nc.dram_tensor("linw2", [128, 4 * 128], f32r, kind="ExternalInput")
    rwb_d = nc.dram_tensor("rwb", [128, 5 * 128], f32r, kind="ExternalInput")
    signs_d = nc.dram_tensor("signs", [128, 1], bf16, kind="ExternalInput")
    ones_d = nc.dram_tensor("ones2", [1, 4 * 512], f32r, kind="ExternalInput")
    out_d = nc.dram_tensor("out", [1, BC], f32, kind="ExternalOutput")

    SQ = mybir.ActivationFunctionType.Square
    CP_ = 512                  # pair-cols per chunk
    CB = CP_ // D              # 8 batches per chunk
    NCH = BC // CB             # 32 chunks
    GROUP = 4                  # chunks per DMA group
    QCH = 8                    # chunks per tail quarter (64 batches)

    with tile.TileContext(nc) as tc, ExitStack() as ctx:
        const = ctx.enter_context(tc.tile_pool(name="const", bufs=1))
        x4p = ctx.enter_context(tc.tile_pool(name="x4p", bufs=2))
        ch4p = ctx.enter_context(tc.tile_pool(name="ch4p", bufs=2))
        chp = ctx.enter_context(tc.tile_pool(name="chp", bufs=8))
        prp = ctx.enter_context(tc.tile_pool(name="prp", bufs=2))
        outp = ctx.enter_context(tc.tile_pool(name="outp", bufs=1))
        zps = ctx.enter_context(tc.tile_pool(name="zps", bufs=5, space="PSUM"))
        yps = ctx.enter_context(tc.tile_pool(name="yps", bufs=2, space="PSUM"))
        qps = ctx.enter_context(tc.tile_pool(name="qps", bufs=1, space="PSUM"))

        linw_t = const.tile([128, 4 * 128], f32r)
        nc.sync.dma_start(linw_t[:], linw_d[:])
        rwb_t = const.tile([128, 5 * 128], f32r)
        nc.sync.dma_start(rwb_t[:], rwb_d[:])
        signs_t = const.tile([128, 1], bf16)
        nc.sync.dma_start(signs_t[:], signs_d[:])
        out_acc = outp.tile([1, BC], f32)
        out_fin = outp.tile([1, BC], f32)

        rep_ctx = tc.For_i(0, repeat, 1) if repeat > 1 else None
        if rep_ctx is not None:
            ctx.enter_context(rep_ctx)

        def stage_a(x4_t, ch4g, cs):
            chs = []
            for j in range(4):
                zt = zps.tile([128, CP_], f32, tag="z")
                nc.tensor.matmul(
                    zt[:],
                    linw_t[32 * j:32 * (j + 1), 128 * j:128 * (j + 1)],
                    x4_t[32 * j:32 * (j + 1), cs],
                    start=True, stop=True,
                    tile_position=(32 * j, 0),
                )
                ch = chp.tile([128, CP_], f32r, tag="ch")
                if j % 2 == 0:
                    nc.scalar.activation(ch[:], zt[:], SQ)
                else:
                    nc.vector._custom_dve(SQUARE_ANT, out=ch[:], in0=zt[:])
                chs.append(ch)
            nc.gpsimd.tensor_mul(ch4g[0:32, cs], x4_t[0:32, cs], x4_t[0:32, cs])
            return (chs, ch4g, cs)

        def stage_b(st, pr_t, pc):
            chs, ch4g, cs = st
            yt = yps.tile([128, CP_], f32, tag="y")
            for j in range(4):
                nc.tensor.matmul(yt[:], rwb_t[:, 128 * j:128 * (j + 1)],
                                 chs[j][:], start=(j == 0), stop=False)
            nc.tensor.matmul(yt[:], rwb_t[0:65, 512:640], ch4g[0:65, cs],
                             start=False, stop=True)
            h = CP_ // 2
            nc.scalar.activation(pr_t[:, pc:pc + h], yt[:, 0:h], SQ)
            nc.vector._custom_dve(SQUARE_ANT, out=pr_t[:, pc + h:pc + CP_],
                                  in0=yt[:, h:CP_])

        NQ = NCH // QCH
        for q in range(NQ):                    # tail quarters (64 batches)
            pr_t = prp.tile([128, QCH * CP_], bf16, tag="pr")
            pend = None
            for gg in range(QCH // GROUP):     # DMA groups within quarter
                g = q * (QCH // GROUP) + gg
                b0 = g * GROUP * CB
                nb = GROUP * CB
                xsrc = x_d[b0:b0 + nb].transpose([1, 0, 2])   # (32, nb, 64)
                x4_t = x4p.tile([128, GROUP * CP_], f32r, tag="x4")
                for j in range(4):
                    nc.sync.dma_start(
                        x4_t[32 * j:32 * (j + 1)].rearrange(
                            "k (b d) -> k b d", b=nb), xsrc)
                ch4g = ch4p.tile([65, GROUP * CP_], f32r, tag="ch4")
                nc.sync.dma_start(
                    ch4g[32:64].rearrange("k (b d) -> k b d", b=nb), xsrc)
                if g < 2:
                    nc.sync.dma_start(ch4g[64:65], ones_d[:])

                for ci in range(GROUP):
                    cs = slice(ci * CP_, (ci + 1) * CP_)
                    c_in_q = gg * GROUP + ci
                    st = stage_a(x4_t, ch4g, cs)
                    if pend is not None:
                        stage_b(*pend)
                    pend = (st, pr_t, c_in_q * CP_)
            stage_b(*pend)
            # tail: 8 accumulating N=512 passes (8-d blocks), then a
            # segmented 8->1 reduce on VectorE into out_acc.
            qacc = qps.tile([1, CP_], f32, tag="qacc")
            pr4 = pr_t[:].rearrange("p (b d) -> p b d", d=D)
            for dd in range(8):
                nc.tensor.matmul(
                    qacc[:], signs_t[:], pr4[:, :, 8 * dd:8 * (dd + 1)],
                    start=(dd == 0), stop=(dd == 7),
                )
            nc.vector.tensor_reduce(
                out_acc[0:1, q * QCH * CB:(q + 1) * QCH * CB],
                qacc[:].rearrange("p (b d) -> p b d", d=8),
                mybir.AxisListType.X, mybir.AluOpType.add,
            )
        nc.scalar.activation(out_fin[:], out_acc[:],
                             mybir.ActivationFunctionType.Copy,
                             bias=float(bconst))
        nc.sync.dma_start(out_d[:], out_fin[:])

    nc.compile()
    _module_cache[key] = nc
    return nc


def build_module4(bconst: float, repeat: int = 1):
    """v4: bf16 z-drain + bf16 big-matmul chains.

    - LIN matmuls drain to bf16 PSUM tiles ([128,1024] = 1 bank for a
      2-chain pair) -> halved PSUM pressure, 16-bit DVE squares.
    - big matmul chains 0-3 in bf16 (FWL weight loads); chain 4 (x^2,x,1)
      and yt accumulation stay f32 for accuracy.
    """
    key = ("v4", round(bconst, 12), repeat)
    if key in _module_cache:
        return _module_cache[key]
    nc = bacc.Bacc("TRN2", target_bir_lowering=False)
    x_d = nc.dram_tensor("x", [BC, F, D], f32r, kind="ExternalInput")
    linw_d = nc.dram_tensor("linw2", [128, 4 * 128], f32r, kind="ExternalInput")
    rwb03_d = nc.dram_tensor("rwb03", [128, 4 * 128], bf16, kind="ExternalInput")
    rwb4_d = nc.dram_tensor("rwb4", [65, 128], f32r, kind="ExternalInput")
    signs_d = nc.dram_tensor("signs", [128, 1], bf16, kind="ExternalInput")
    ones_d = nc.dram_tensor("ones2", [1, 4 * 512], f32r, kind="ExternalInput")
    out_d = nc.dram_tensor("out", [1, BC], f32, kind="ExternalOutput")

    SQ = mybir.ActivationFunctionType.Square
    CP_ = 512                  # pair-cols per chunk
    CB = CP_ // D              # 8 batches per chunk
    NCH = BC // CB             # 32 chunks
    GROUP = 4                  # chunks per DMA group
    QCH = 8                    # chunks per tail quarter (64 batches)

    with tile.TileContext(nc) as tc, ExitStack() as ctx:
        const = ctx.enter_context(tc.tile_pool(name="const", bufs=1))
        x4p = ctx.enter_context(tc.tile_pool(name="x4p", bufs=2))
        ch4p = ctx.enter_context(tc.tile_pool(name="ch4p", bufs=2))
        chp = ctx.enter_context(tc.tile_pool(name="chp", bufs=4))
        prp = ctx.enter_context(tc.tile_pool(name="prp", bufs=2))
        outp = ctx.enter_context(tc.tile_pool(name="outp", bufs=1))
        zpsA = ctx.enter_context(tc.tile_pool(name="zpsA", bufs=1, space="PSUM"))
        zpsB = ctx.enter_context(tc.tile_pool(name="zpsB", bufs=1, space="PSUM"))
        yps = ctx.enter_context(tc.tile_pool(name="yps", bufs=2, space="PSUM"))
        qps = ctx.enter_context(tc.tile_pool(name="qps", bufs=2, space="PSUM"))

        linw_t = const.tile([128, 4 * 128], f32r)
        nc.sync.dma_start(linw_t[:], linw_d[:])
        rwb03_t = const.tile([128, 4 * 128], bf16)
        nc.sync.dma_start(rwb03_t[:], rwb03_d[:])
        rwb4_t = const.tile([65, 128], f32r)
        nc.sync.dma_start(rwb4_t[:], rwb4_d[:])
        signs_t = const.tile([128, 1], bf16)
        nc.sync.dma_start(signs_t[:], signs_d[:])
        out_acc = outp.tile([1, BC], f32)
        out_fin = outp.tile([1, BC], f32)

        rep_ctx = tc.For_i(0, repeat, 1) if repeat > 1 else None
        if rep_ctx is not None:
            ctx.enter_context(rep_ctx)

        def stage_a(x4_t, ch4g, cs):
            zA = zpsA.tile([128, 2 * CP_], f32, tag="zA")
            zB = zpsB.tile([128, 2 * CP_], f32, tag="zB")
            for j, (zt, off) in enumerate(
                    [(zA, 0), (zA, CP_), (zB, 0), (zB, CP_)]):
                nc.tensor.matmul(
                    zt[:, off:off + CP_],
                    linw_t[32 * j:32 * (j + 1), 128 * j:128 * (j + 1)],
                    x4_t[32 * j:32 * (j + 1), cs],
                    start=True, stop=True,
                    tile_position=(32 * j, 0),
                )
            chA = chp.tile([128, 2 * CP_], bf16, tag="chA")
            chB = chp.tile([128, 2 * CP_], bf16, tag="chB")
            nc.scalar.activation(chA[:], zA[:], SQ)
            nc.vector._custom_dve(SQUARE_ANT, out=chB[:], in0=zB[:])
            nc.gpsimd.tensor_mul(ch4g[0:32, cs], x4_t[0:32, cs], x4_t[0:32, cs])
            return (chA, chB, ch4g, cs)

        def stage_b(st, pr_t, pc):
            chA, chB, ch4g, cs = st
            yt = yps.tile([128, CP_], f32, tag="y")
            rhs = [chA[:, 0:CP_], chA[:, CP_:2 * CP_],
                   chB[:, 0:CP_], chB[:, CP_:2 * CP_]]
            for j in range(4):
                nc.tensor.matmul(yt[:], rwb03_t[:, 128 * j:128 * (j + 1)],
                                 rhs[j], start=(j == 0), stop=False)
            nc.tensor.matmul(yt[:], rwb4_t[:], ch4g[0:65, cs],
                             start=False, stop=True)
            h = CP_ // 2
            nc.scalar.activation(pr_t[:, pc:pc + h], yt[:, 0:h], SQ)
            nc.vector._custom_dve(SQUARE_ANT, out=pr_t[:, pc + h:pc + CP_],
                                  in0=yt[:, h:CP_])

        NQ = NCH // QCH
        for q in range(NQ):                    # tail quarters (64 batches)
            pr_t = prp.tile([128, QCH * CP_], bf16, tag="pr")
            pend = None
            for gg in range(QCH // GROUP):     # DMA groups within quarter
                g = q * (QCH // GROUP) + gg
                b0 = g * GROUP * CB
                nb = GROUP * CB
                xsrc = x_d[b0:b0 + nb].transpose([1, 0, 2])   # (32, nb, 64)
                x4_t = x4p.tile([128, GROUP * CP_], f32r, tag="x4")
                for j in range(4):
                    nc.sync.dma_start(
                        x4_t[32 * j:32 * (j + 1)].rearrange(
                            "k (b d) -> k b d", b=nb), xsrc)
                ch4g = ch4p.tile([65, GROUP * CP_], f32r, tag="ch4")
                nc.sync.dma_start(
                    ch4g[32:64].rearrange("k (b d) -> k b d", b=nb), xsrc)
                if g < 2:
                    nc.sync.dma_start(ch4g[64:65], ones_d[:])

                for ci in range(GROUP):
                    cs = slice(ci * CP_, (ci + 1) * CP_)
                    c_in_q = gg * GROUP + ci
                    st = stage_a(x4_t, ch4g, cs)
                    if pend is not None:
                        stage_b(*pend)
                    pend = (st, pr_t, c_in_q * CP_)
            stage_b(*pend)
            qacc = qps.tile([1, CP_], f32, tag="qacc")
            pr4 = pr_t[:].rearrange("p (b d) -> p b d", d=D)
            for dd in range(8):
                nc.tensor.matmul(
                    qacc[:], signs_t[:], pr4[:, :, 8 * dd:8 * (dd + 1)],
                    start=(dd == 0), stop=(dd == 7),
                )
            nc.vector.tensor_reduce(
                out_acc[0:1, q * QCH * CB:(q + 1) * QCH * CB],
                qacc[:].rearrange("p (b d) -> p b d", d=8),
                mybir.AxisListType.X, mybir.AluOpType.add,
            )
        nc.scalar.activation(out_fin[:], out_acc[:],
                             mybir.ActivationFunctionType.Copy,
                             bias=float(bconst))
        nc.sync.dma_start(out_d[:], out_fin[:])

    nc.compile()
    _module_cache[key] = nc
    return nc


def build_module5(bconst: float, repeat: int = 1):
    """v5: PSUM bank recycling for deep PE pipelining.

    - ringA [128,1024] f32 x2: chains 0,1 z + (after the square reads them)
      the yt accumulator reuses cols 0:512 of the same banks (WAR dep).
    - ringB/ringC [128,512] x2: chains 2,3; qacc tag-shares ringB.
    - squares: scalar = sqA (1024 cols) + pr (512); vector = sqB + sqC (512
      each) -> vector FIFO never waits on stage_b.
    """
    key = ("v5", round(bconst, 12), repeat)
    if key in _module_cache:
        return _module_cache[key]
    nc = bacc.Bacc("TRN2", target_bir_lowering=False)
    x_d = nc.dram_tensor("x", [BC, F, D], f32r, kind="ExternalInput")
    linw_d = nc.dram_tensor("linw2", [128, 4 * 128], f32r, kind="ExternalInput")
    rwb03_d = nc.dram_tensor("rwb03", [128, 4 * 128], bf16, kind="ExternalInput")
    rwb4_d = nc.dram_tensor("rwb4", [65, 128], f32r, kind="ExternalInput")
    signs_d = nc.dram_tensor("signs", [128, 1], bf16, kind="ExternalInput")
    ones_d = nc.dram_tensor("ones2", [1, 4 * 512], f32r, kind="ExternalInput")
    out_d = nc.dram_tensor("out", [1, BC], f32, kind="ExternalOutput")

    SQ = mybir.ActivationFunctionType.Square
    CP_ = 512
    CB = CP_ // D              # 8 batches per chunk
    NCH = BC // CB             # 32 chunks
    GROUP = 4
    QCH = 8

    with tile.TileContext(nc) as tc, ExitStack() as ctx:
        const = ctx.enter_context(tc.tile_pool(name="const", bufs=1))
        x4p = ctx.enter_context(tc.tile_pool(name="x4p", bufs=2))
        ch4p = ctx.enter_context(tc.tile_pool(name="ch4p", bufs=2))
        chp = ctx.enter_context(tc.tile_pool(name="chp", bufs=2))
        prp = ctx.enter_context(tc.tile_pool(name="prp", bufs=2))
        outp = ctx.enter_context(tc.tile_pool(name="outp", bufs=1))
        ringA = ctx.enter_context(tc.tile_pool(name="ringA", bufs=2, space="PSUM"))
        ringB = ctx.enter_context(tc.tile_pool(name="ringB", bufs=2, space="PSUM"))
        ringC = ctx.enter_context(tc.tile_pool(name="ringC", bufs=2, space="PSUM"))

        linw_t = const.tile([128, 4 * 128], f32r)
        nc.sync.dma_start(linw_t[:], linw_d[:])
        rwb03_t = const.tile([128, 4 * 128], bf16)
        nc.sync.dma_start(rwb03_t[:], rwb03_d[:])
        rwb4_t = const.tile([65, 128], f32r)
        nc.sync.dma_start(rwb4_t[:], rwb4_d[:])
        signs_t = const.tile([128, 1], bf16)
        nc.sync.dma_start(signs_t[:], signs_d[:])
        out_acc = outp.tile([1, BC], f32)
        out_fin = outp.tile([1, BC], f32)

        rep_ctx = tc.For_i(0, repeat, 1) if repeat > 1 else None
        if rep_ctx is not None:
            ctx.enter_context(rep_ctx)

        def stage_a(x4_t, ch4g, cs):
            zA = ringA.tile([128, 2 * CP_], f32, tag="zA")
            zB = ringB.tile([128, CP_], f32, tag="zB")
            zC = ringC.tile([128, CP_], f32, tag="zC")
            for j, (dst, off) in enumerate(
                    [(zA, 0), (zA, CP_), (zB, 0), (zC, 0)]):
                nc.tensor.matmul(
                    dst[:, off:off + CP_],
                    linw_t[32 * j:32 * (j + 1), 128 * j:128 * (j + 1)],
                    x4_t[32 * j:32 * (j + 1), cs],
                    start=True, stop=True,
                    tile_position=(32 * j, 0),
                )
            chA = chp.tile([128, 2 * CP_], bf16, tag="chA")
            chB = chp.tile([128, CP_], bf16, tag="chB")
            chC = chp.tile([128, CP_], bf16, tag="chC")
            nc.scalar.activation(chA[:], zA[:], SQ)
            nc.vector._custom_dve(SQUARE_ANT, out=chB[:], in0=zB[:])
            nc.vector._custom_dve(SQUARE_ANT, out=chC[:], in0=zC[:])
            nc.gpsimd.tensor_mul(ch4g[0:32, cs], x4_t[0:32, cs], x4_t[0:32, cs])
            return (chA, chB, chC, ch4g, cs, zA)

        def stage_b(st, pr_t, pc):
            chA, chB, chC, ch4g, cs, zA = st
            yt = zA[:, 0:CP_]          # recycle bank 0 of this chunk's zA
            rhs = [chA[:, 0:CP_], chA[:, CP_:2 * CP_], chB[:], chC[:]]
            for j in range(4):
                nc.tensor.matmul(yt, rwb03_t[:, 128 * j:128 * (j + 1)],
                                 rhs[j], start=(j == 0), stop=False)
            nc.tensor.matmul(yt, rwb4_t[:], ch4g[0:65, cs],
                             start=False, stop=True)
            nc.scalar.activation(pr_t[:, pc:pc + CP_], yt, SQ)

        NQ = NCH // QCH
        for q in range(NQ):
            pr_t = prp.tile([128, QCH * CP_], bf16, tag="pr")
            pend = None
            for gg in range(QCH // GROUP):
                g = q * (QCH // GROUP) + gg
                b0 = g * GROUP * CB
                nb = GROUP * CB
                xsrc = x_d[b0:b0 + nb].transpose([1, 0, 2])   # (32, nb, 64)
                x4_t = x4p.tile([128, GROUP * CP_], f32r, tag="x4")
                for j in range(4):
                    nc.sync.dma_start(
                        x4_t[32 * j:32 * (j + 1)].rearrange(
                            "k (b d) -> k b d", b=nb), xsrc)
                ch4g = ch4p.tile([65, GROUP * CP_], f32r, tag="ch4")
                nc.sync.dma_start(
                    ch4g[32:64].rearrange("k (b d) -> k b d", b=nb), xsrc)
                if g < 2:
                    nc.sync.dma_start(ch4g[64:65], ones_d[:])

                for ci in range(GROUP):
                    cs = slice(ci * CP_, (ci + 1) * CP_)
                    c_in_q = gg * GROUP + ci
                    st = stage_a(x4_t, ch4g, cs)
                    if pend is not None:
                        stage_b(*pend)
                    pend = (st, pr_t, c_in_q * CP_)
            stage_b(*pend)
            qacc = ringB.tile([1, CP_], f32, tag="zB")
            pr4 = pr_t[:].rearrange("p (b d) -> p b d", d=D)
            for dd in range(8):
                nc.tensor.matmul(
                    qacc[:], signs_t[:], pr4[:, :, 8 * dd:8 * (dd + 1)],
                    start=(dd == 0), stop=(dd == 7),
                )
            nc.vector.tensor_reduce(
                out_acc[0:1, q * QCH * CB:(q + 1) * QCH * CB],
                qacc[:].rearrange("p (b d) -> p b d", d=8),
                mybir.AxisListType.X, mybir.AluOpType.add,
            )
        nc.scalar.activation(out_fin[:], out_acc[:],
                             mybir.ActivationFunctionType.Copy,
                             bias=float(bconst))
        nc.sync.dma_start(out_d[:], out_fin[:])

    nc.compile()
    _module_cache[key] = nc
    return nc


def build_module6(bconst: float, repeat: int = 1, warmup: int = 14):
    """v6: v5 + PE warmup burst (trip HAM to 2.4GHz during the DMA lead-in)
    + quarter tails interleaved into the next quarter's chunk pipeline so
    scalar/vector queues never drain at quarter boundaries."""
    key = ("v6", round(bconst, 12), repeat, warmup)
    if key in _module_cache:
        return _module_cache[key]
    nc = bacc.Bacc("TRN2", target_bir_lowering=False)
    x_d = nc.dram_tensor("x", [BC, F, D], f32r, kind="ExternalInput")
    linw_d = nc.dram_tensor("linw2", [128, 4 * 128], f32r, kind="ExternalInput")
    rwb03_d = nc.dram_tensor("rwb03", [128, 4 * 128], bf16, kind="ExternalInput")
    rwb4_d = nc.dram_tensor("rwb4", [65, 128], f32r, kind="ExternalInput")
    signs_d = nc.dram_tensor("signs", [128, 1], bf16, kind="ExternalInput")
    ones_d = nc.dram_tensor("ones2", [1, 4 * 512], f32r, kind="ExternalInput")
    out_d = nc.dram_tensor("out", [1, BC], f32, kind="ExternalOutput")

    SQ = mybir.ActivationFunctionType.Square
    CP_ = 512
    CB = CP_ // D
    NCH = BC // CB             # 32 chunks
    GROUP = 4
    QCH = 8

    with tile.TileContext(nc) as tc, ExitStack() as ctx:
        const = ctx.enter_context(tc.tile_pool(name="const", bufs=1))
        x4p = ctx.enter_context(tc.tile_pool(name="x4p", bufs=2))
        ch4p = ctx.enter_context(tc.tile_pool(name="ch4p", bufs=2))
        chp = ctx.enter_context(tc.tile_pool(name="chp", bufs=2))
        prp = ctx.enter_context(tc.tile_pool(name="prp", bufs=2))
        outp = ctx.enter_context(tc.tile_pool(name="outp", bufs=1))
        ringA = ctx.enter_context(tc.tile_pool(name="ringA", bufs=2, space="PSUM"))
        ringB = ctx.enter_context(tc.tile_pool(name="ringB", bufs=2, space="PSUM"))
        ringC = ctx.enter_context(tc.tile_pool(name="ringC", bufs=2, space="PSUM"))

        linw_t = const.tile([128, 4 * 128], f32r)
        nc.sync.dma_start(linw_t[:], linw_d[:])
        rwb03_t = const.tile([128, 4 * 128], bf16)
        nc.sync.dma_start(rwb03_t[:], rwb03_d[:])
        rwb4_t = const.tile([65, 128], f32r)
        nc.sync.dma_start(rwb4_t[:], rwb4_d[:])
        signs_t = const.tile([128, 1], bf16)
        nc.sync.dma_start(signs_t[:], signs_d[:])
        out_acc = outp.tile([1, BC], f32)
        out_fin = outp.tile([1, BC], f32)

        rep_ctx = tc.For_i(0, repeat, 1) if repeat > 1 else None
        if rep_ctx is not None:
            ctx.enter_context(rep_ctx)

        # --- PE warmup: serial K=128 N=512 matmuls while group-0 DMA lands.
        wps = None
        if warmup:
            wps = ringA.tile([128, 2 * CP_], f32, tag="zA", name="warm")
            for w in range(warmup):
                nc.tensor.matmul(wps[:, 0:CP_], linw_t[:, 0:128],
                                 linw_t[:, 0:CP_], start=True, stop=True)

        def stage_a(x4_t, ch4g, cs, fill=0):
            zA = ringA.tile([128, 2 * CP_], f32, tag="zA")
            # filler matmuls keep HAM warm across dependency stalls; the real
            # chain-0 matmul below overwrites them (start=True clears bank).
            for w in range(fill):
                nc.tensor.matmul(zA[0:1, 0:CP_], signs_t[:],
                                 rwb03_t[:, 0:CP_], start=True, stop=True)
            zB = ringB.tile([128, CP_], f32, tag="zB")
            zC = ringC.tile([128, CP_], f32, tag="zC")
            for j, (dst, off) in enumerate(
                    [(zA, 0), (zA, CP_), (zB, 0), (zC, 0)]):
                nc.tensor.matmul(
                    dst[:, off:off + CP_],
                    linw_t[32 * j:32 * (j + 1), 128 * j:128 * (j + 1)],
                    x4_t[32 * j:32 * (j + 1), cs],
                    start=True, stop=True,
                    tile_position=(32 * j, 0),
                )
            chA = chp.tile([128, 2 * CP_], bf16, tag="chA")
            chB = chp.tile([128, CP_], bf16, tag="chB")
            chC = chp.tile([128, CP_], bf16, tag="chC")
            nc.scalar.activation(chA[:], zA[:], SQ)
            nc.vector._custom_dve(SQUARE_ANT, out=chB[:], in0=zB[:])
            nc.vector._custom_dve(SQUARE_ANT, out=chC[:], in0=zC[:])
            nc.gpsimd.tensor_mul(ch4g[0:32, cs], x4_t[0:32, cs], x4_t[0:32, cs])
            return (chA, chB, chC, ch4g, cs, zA)

        def stage_b(st, pr_t, pc):
            chA, chB, chC, ch4g, cs, zA = st
            yt = zA[:, 0:CP_]
            rhs = [chA[:, 0:CP_], chA[:, CP_:2 * CP_], chB[:], chC[:]]
            for j in range(4):
                nc.tensor.matmul(yt, rwb03_t[:, 128 * j:128 * (j + 1)],
                                 rhs[j], start=(j == 0), stop=False)
            nc.tensor.matmul(yt, rwb4_t[:], ch4g[0:65, cs],
                             start=False, stop=True)
            nc.scalar.activation(pr_t[:, pc:pc + CP_], yt, SQ)

        def tail(pr_t, q):
            qacc = ringB.tile([1, CP_], f32, tag="zB", name="qacc")
            pr4 = pr_t[:].rearrange("p (b d) -> p b d", d=D)
            for dd in range(8):
                nc.tensor.matmul(
                    qacc[:], signs_t[:], pr4[:, :, 8 * dd:8 * (dd + 1)],
                    start=(dd == 0), stop=(dd == 7),
                )
            nc.vector.tensor_reduce(
                out_acc[0:1, q * QCH * CB:(q + 1) * QCH * CB],
                qacc[:].rearrange("p (b d) -> p b d", d=8),
                mybir.AxisListType.X, mybir.AluOpType.add,
            )

        pend = None
        pend_tail = None           # (pr_t, q) awaiting tail issue
        pr_tiles = {}
        for c in range(NCH):
            q = c // QCH
            if c % QCH == 0:
                pr_tiles[q] = prp.tile([128, QCH * CP_], bf16, tag="pr",
                                       name=f"pr{q}")
            if c % GROUP == 0:
                g = c // GROUP
                b0 = g * GROUP * CB
                nb = GROUP * CB
                xsrc = x_d[b0:b0 + nb].transpose([1, 0, 2])   # (32, nb, 64)
                x4_t = x4p.tile([128, GROUP * CP_], f32r, tag="x4")
                for j in range(4):
                    nc.sync.dma_start(
                        x4_t[32 * j:32 * (j + 1)].rearrange(
                            "k (b d) -> k b d", b=nb), xsrc)
                ch4g = ch4p.tile([65, GROUP * CP_], f32r, tag="ch4")
                nc.sync.dma_start(
                    ch4g[32:64].rearrange("k (b d) -> k b d", b=nb), xsrc)
                if g < 2:
                    nc.sync.dma_start(ch4g[64:65], ones_d[:])

            cs = slice((c % GROUP) * CP_, (c % GROUP + 1) * CP_)
            st = stage_a(x4_t, ch4g, cs, fill=3 if c < 4 else 1)
            if pend is not None:
                stage_b(*pend)
            if pend_tail is not None and c >= pend_tail[0]:
                tail(*pend_tail[1:])
                pend_tail = None
            pend = (st, pr_tiles[q], (c % QCH) * CP_)
            if c % QCH == QCH - 1:
                pend_tail = (c + 2, pr_tiles[q], q)
        stage_b(*pend)
        if pend_tail is not None:
            tail(*pend_tail[1:])
        nc.scalar.activation(out_fin[:], out_acc[:],
                             mybir.ActivationFunctionType.Copy,
                             bias=float(bconst))
        nc.sync.dma_start(out_d[:], out_fin[:])

    nc.compile()
    _module_cache[key] = nc
    return nc


CFG7 = {"xbf": True, "warmup": 11}


def build_module7(bconst: float, repeat: int = 1):
    """v7: v2's proven-concurrent PSUM layout (zA/zB pairs bufs=1, yt bufs=2,
    qacc bufs=2) + bf16 LIN (exact 0/1 weights, FWL) + bf16 x for the z path
    (chain 4 stays f32r via the separate ch4g load) + warmup + interleaved
    tails."""
    xbf = CFG7["xbf"]
    warmup = CFG7["warmup"]
    key = ("v7", round(bconst, 12), repeat, xbf, warmup)
    if key in _module_cache:
        return _module_cache[key]
    nc = bacc.Bacc("TRN2", target_bir_lowering=False)
    xdt = bf16 if xbf else f32r
    x_d = nc.dram_tensor("x", [F, BC, D], f32r, kind="ExternalInput")
    xb_d = (nc.dram_tensor("xb", [F, BC, D], bf16, kind="ExternalInput")
            if xbf else x_d)
    linw_d = nc.dram_tensor("linw2", [128, 4 * 128], xdt, kind="ExternalInput")
    rwb03_d = nc.dram_tensor("rwb03", [128, 4 * 128], bf16, kind="ExternalInput")
    rwb4_d = nc.dram_tensor("rwb4", [65, 128], f32r, kind="ExternalInput")
    signs_d = nc.dram_tensor("signs", [128, 1], bf16, kind="ExternalInput")
    ones_d = nc.dram_tensor("ones2", [1, 32 * 512], f32r, kind="ExternalInput")
    out_d = nc.dram_tensor("out", [1, BC], f32, kind="ExternalOutput")

    SQ = mybir.ActivationFunctionType.Square
    CP_ = 512
    CB = CP_ // D
    NCH = BC // CB
    GROUP = 16
    QCH = 8

    with tile.TileContext(nc) as tc, ExitStack() as ctx:
        const = ctx.enter_context(tc.tile_pool(name="const", bufs=1))
        x4p = ctx.enter_context(tc.tile_pool(name="x4p", bufs=1))
        ch4p = ctx.enter_context(tc.tile_pool(name="ch4p", bufs=1))
        chp = ctx.enter_context(tc.tile_pool(name="chp", bufs=2))
        prp = ctx.enter_context(tc.tile_pool(name="prp", bufs=2))
        outp = ctx.enter_context(tc.tile_pool(name="outp", bufs=1))
        zpsA = ctx.enter_context(tc.tile_pool(name="zpsA", bufs=1, space="PSUM"))
        zpsB = ctx.enter_context(tc.tile_pool(name="zpsB", bufs=2, space="PSUM"))
        yps = ctx.enter_context(tc.tile_pool(name="yps", bufs=2, space="PSUM"))

        linw_t = const.tile([128, 4 * 128], xdt)
        nc.sync.dma_start(linw_t[:], linw_d[:])
        rwb03_t = const.tile([128, 4 * 128], bf16)
        nc.sync.dma_start(rwb03_t[:], rwb03_d[:])
        rwb4_t = const.tile([65, 128], f32r)
        nc.sync.dma_start(rwb4_t[:], rwb4_d[:])
        signs_t = const.tile([128, 1], bf16)
        nc.sync.dma_start(signs_t[:], signs_d[:])
        out_acc = outp.tile([1, BC], f32)
        out_fin = outp.tile([1, BC], f32)

        rep_ctx = tc.For_i(0, repeat, 1) if repeat > 1 else None
        if rep_ctx is not None:
            ctx.enter_context(rep_ctx)

        if warmup:
            wq = yps.tile([128, CP_], f32, tag="y", name="warm")
            for w in range(warmup):
                nc.tensor.matmul(wq[:], rwb03_t[:, 0:128], rwb03_t[:, 0:CP_],
                                 start=True, stop=True)

        def stage_a(x4_t, ch4g, cs):
            zA = zpsA.tile([128, 2 * CP_], f32, tag="zA")
            zB = zpsB.tile([128, 2 * CP_], f32, tag="zB")
            for j, (zt, off) in enumerate(
                    [(zA, 0), (zA, CP_), (zB, 0), (zB, CP_)]):
                nc.tensor.matmul(
                    zt[:, off:off + CP_],
                    linw_t[32 * j:32 * (j + 1), 128 * j:128 * (j + 1)],
                    x4_t[32 * j:32 * (j + 1), cs],
                    start=True, stop=True,
                    tile_position=(32 * j, 0),
                )
            chA = chp.tile([128, 2 * CP_], bf16, tag="chA")
            chB = chp.tile([128, 2 * CP_], bf16, tag="chB")
            nc.scalar.activation(chA[:], zA[:], SQ)
            nc.vector._custom_dve(SQUARE_ANT, out=chB[:], in0=zB[:])
            nc.gpsimd.tensor_mul(ch4g[0:32, cs], ch4g[32:64, cs],
                                 ch4g[32:64, cs])
            return (chA, chB, ch4g, cs)

        def stage_b1(st):
            chA, chB, ch4g, cs = st
            yt = yps.tile([128, CP_], f32, tag="y")
            rhs = [chA[:, 0:CP_], chA[:, CP_:2 * CP_],
                   chB[:, 0:CP_], chB[:, CP_:2 * CP_]]
            for j in range(4):
                nc.tensor.matmul(yt[:], rwb03_t[:, 128 * j:128 * (j + 1)],
                                 rhs[j], start=(j == 0), stop=False)
            nc.tensor.matmul(yt[:], rwb4_t[:], ch4g[0:65, cs],
                             start=False, stop=True)
            return yt

        def stage_b2(yt, pr_t, pc):
            nc.scalar.activation(pr_t[:, pc:pc + CP_], yt[:], SQ)

        def tail(pr_t, q):
            qacc = yps.tile([1, CP_], f32, tag="y")
            pr4 = pr_t[:].rearrange("p (b d) -> p b d", d=D)
            for dd in range(8):
                nc.tensor.matmul(
                    qacc[:], signs_t[:], pr4[:, :, 8 * dd:8 * (dd + 1)],
                    start=(dd == 0), stop=(dd == 7),
                )
            return qacc

        def tail_red(qacc, q):
            nc.vector.tensor_reduce(
                out_acc[0:1, q * QCH * CB:(q + 1) * QCH * CB],
                qacc[:].rearrange("p (b d) -> p b d", d=8),
                mybir.AxisListType.X, mybir.AluOpType.add,
            )

        pend = None            # stage_a result awaiting big matmuls (dist 1)
        pend2 = None           # yt awaiting pr square (dist 2)
        pend_tail = None
        pend_red = None
        pr_tiles = {}
        for c in range(NCH):
            q = c // QCH
            if c % QCH == 0:
                pr_tiles[q] = prp.tile([128, QCH * CP_], bf16, tag="pr",
                                       name=f"pr{q}")
            if c in (0,):              # single group, finely sliced
                gsz = 32
                gstart = c
                b0 = c * CB
                nb = gsz * CB
                nsl = 8                    # slices per group
                nh = nb // nsl
                x4_t = x4p.tile([128, gsz * CP_], xdt, tag="x4")
                ch4g = ch4p.tile([65, gsz * CP_], f32r, tag="ch4")
                for h in range(nsl):
                    hb = b0 + h * nh
                    hc = slice(h * nh * D, (h + 1) * nh * D)
                    xsrc = xb_d[:, hb:hb + nh, :]
                    for j in range(4):
                        nc.sync.dma_start(
                            x4_t[32 * j:32 * (j + 1), hc].rearrange(
                                "k (b d) -> k b d", b=nh), xsrc)
                    nc.sync.dma_start(
                        ch4g[32:64, hc].rearrange("k (b d) -> k b d", b=nh),
                        x_d[:, hb:hb + nh, :])
                nc.sync.dma_start(ch4g[64:65], ones_d[0:1, 0:gsz * 512])

            cs = slice((c - gstart) * CP_, (c - gstart + 1) * CP_)
            if pend2 is not None:
                stage_b2(*pend2)
                pend2 = None
            st = stage_a(x4_t, ch4g, cs)
            if pend is not None:
                yt = stage_b1(pend[0])
                pend2 = (yt, pend[1], pend[2])
            if pend_tail is not None and c >= pend_tail[0]:
                tail_red(tail(*pend_tail[1:]), pend_tail[2])
                pend_tail = None
            pend = (st, pr_tiles[q], (c % QCH) * CP_)
            if c % QCH == QCH - 1:
                pend_tail = (c + 3, pr_tiles[q], q)
        yt = stage_b1(pend[0])
        stage_b2(*pend2)
        stage_b2(yt, pend[1], pend[2])
        if pend_tail is not None:
            tail_red(tail(*pend_tail[1:]), pend_tail[2])
        nc.scalar.activation(out_fin[:], out_acc[:],
                             mybir.ActivationFunctionType.Copy,
                             bias=float(bconst))
        nc.sync.dma_start(out_d[:], out_fin[:])

    nc.compile()
    _module_cache[key] = nc
    return nc


def _run7(inputs, trace=False, **kw):
    folded, bconst, _info = fold_weights2(
        inputs["W1"], inputs["b1"], inputs["W2"], inputs["b2"],
        inputs["W3"], inputs["b3"], inputs["W_out"], inputs["b_out"],
    )
    import ml_dtypes
    nc = build_module7(bconst)
    x0 = np.ascontiguousarray(np.asarray(inputs["x0"], dtype=np.float32))
    rwb = folded["rwb"]
    in_maps = []
    for c in range(NCORES):
        xs = np.ascontiguousarray(x0[BC * c:BC * (c + 1)])
        xt = np.ascontiguousarray(xs.transpose(1, 0, 2))   # (F, BC, D)
        m = {
            "linw2": folded["linw2"].astype(
                ml_dtypes.bfloat16 if CFG7["xbf"] else np.float32),
            "rwb03": np.ascontiguousarray(rwb[:, 0:512]).astype(ml_dtypes.bfloat16),
            "rwb4": np.ascontiguousarray(rwb[0:65, 512:640]),
            "signs": folded["signs"].astype(ml_dtypes.bfloat16),
            "ones2": np.ones((1, 32 * 512), dtype=np.float32),
            "x": xt,
        }
        if CFG7["xbf"]:
            m["xb"] = xt.astype(ml_dtypes.bfloat16)
        in_maps.append(m)
    res = run_bass_kernel_spmd(nc, in_maps, core_ids=list(range(NCORES)),
                               trace=trace, **kw)
    out = np.concatenate(
        [res.results[c]["out"].reshape(BC, 1) for c in range(NCORES)], axis=0
    )
    return out, res


def _run6(inputs, trace=False, **kw):
    folded, bconst, _info = fold_weights2(
        inputs["W1"], inputs["b1"], inputs["W2"], inputs["b2"],
        inputs["W3"], inputs["b3"], inputs["W_out"], inputs["b_out"],
    )
    import ml_dtypes
    nc = build_module6(bconst)
    x0 = np.ascontiguousarray(np.asarray(inputs["x0"], dtype=np.float32))
    rwb = folded["rwb"]
    in_maps = []
    for c in range(NCORES):
        m = {
            "linw2": folded["linw2"],
            "rwb03": np.ascontiguousarray(rwb[:, 0:512]).astype(ml_dtypes.bfloat16),
            "rwb4": np.ascontiguousarray(rwb[0:65, 512:640]),
            "signs": folded["signs"].astype(ml_dtypes.bfloat16),
            "ones2": np.ones((1, 4 * 512), dtype=np.float32),
            "x": np.ascontiguousarray(x0[BC * c:BC * (c + 1)]),
        }
        in_maps.append(m)
    res = run_bass_kernel_spmd(nc, in_maps, core_ids=list(range(NCORES)),
                               trace=trace, **kw)
    out = np.concatenate(
        [res.results[c]["out"].reshape(BC, 1) for c in range(NCORES)], axis=0
    )
    return out, res


def _run5(inputs, trace=False, **kw):
    folded, bconst, _info = fold_weights2(
        inputs["W1"], inputs["b1"], inputs["W2"], inputs["b2"],
        inputs["W3"], inputs["b3"], inputs["W_out"], inputs["b_out"],
    )
    import ml_dtypes
    nc = build_module5(bconst)
    x0 = np.ascontiguousarray(np.asarray(inputs["x0"], dtype=np.float32))
    rwb = folded["rwb"]
    in_maps = []
    for c in range(NCORES):
        m = {
            "linw2": folded["linw2"],
            "rwb03": np.ascontiguousarray(rwb[:, 0:512]).astype(ml_dtypes.bfloat16),
            "rwb4": np.ascontiguousarray(rwb[0:65, 512:640]),
            "signs": folded["signs"].astype(ml_dtypes.bfloat16),
            "ones2": np.ones((1, 4 * 512), dtype=np.float32),
            "x": np.ascontiguousarray(x0[BC * c:BC * (c + 1)]),
        }
        in_maps.append(m)
    res = run_bass_kernel_spmd(nc, in_maps, core_ids=list(range(NCORES)),
                               trace=trace, **kw)
    out = np.concatenate(
        [res.results[c]["out"].reshape(BC, 1) for c in range(NCORES)], axis=0
    )
    return out, res


def _run4(inputs, trace=False, **kw):
    folded, bconst, _info = fold_weights2(
        inputs["W1"], inputs["b1"], inputs["W2"], inputs["b2"],
        inputs["W3"], inputs["b3"], inputs["W_out"], inputs["b_out"],
    )
    import ml_dtypes
    nc = build_module4(bconst)
    x0 = np.ascontiguousarray(np.asarray(inputs["x0"], dtype=np.float32))
    rwb = folded["rwb"]
    in_maps = []
    for c in range(NCORES):
        m = {
            "linw2": folded["linw2"],
            "rwb03": np.ascontiguousarray(rwb[:, 0:512]).astype(ml_dtypes.bfloat16),
            "rwb4": np.ascontiguousarray(rwb[0:65, 512:640]),
            "signs": folded["signs"].astype(ml_dtypes.bfloat16),
            "ones2": np.ones((1, 4 * 512), dtype=np.float32),
            "x": np.ascontiguousarray(x0[BC * c:BC * (c + 1)]),
        }
        in_maps.append(m)
    res = run_bass_kernel_spmd(nc, in_maps, core_ids=list(range(NCORES)),
                               trace=trace, **kw)
    out = np.concatenate(
        [res.results[c]["out"].reshape(BC, 1) for c in range(NCORES)], axis=0
    )
    return out, res


def _run3(inputs, trace=False, **kw):
    folded, bconst, _info = fold_weights2(
        inputs["W1"], inputs["b1"], inputs["W2"], inputs["b2"],
        inputs["W3"], inputs["b3"], inputs["W_out"], inputs["b_out"],
    )
    import ml_dtypes
    nc = build_module3(bconst)
    x0 = np.ascontiguousarray(np.asarray(inputs["x0"], dtype=np.float32))
    in_maps = []
    for c in range(NCORES):
        m = {
            "linw2": folded["linw2"],
            "rwb": folded["rwb"],
            "signs": folded["signs"].astype(ml_dtypes.bfloat16),
            "ones2": np.ones((1, 4 * 512), dtype=np.float32),
            "x": np.ascontiguousarray(x0[BC * c:BC * (c + 1)]),
        }
        in_maps.append(m)
    res = run_bass_kernel_spmd(nc, in_maps, core_ids=list(range(NCORES)),
                               trace=trace, **kw)
    out = np.concatenate(
        [res.results[c]["out"].reshape(BC, 1) for c in range(NCORES)], axis=0
    )
    return out, res


def _run2(inputs, trace=False, **kw):
    folded, bconst, _info = fold_weights2(
        inputs["W1"], inputs["b1"], inputs["W2"], inputs["b2"],
        inputs["W3"], inputs["b3"], inputs["W_out"], inputs["b_out"],
    )
    import ml_dtypes
    nc = build_module2(bconst)
    x0 = np.ascontiguousarray(np.asarray(inputs["x0"], dtype=np.float32))
    in_maps = []
    for c in range(NCORES):
        m = {
            "linw2": folded["linw2"],
            "rwb": folded["rwb"],
            "signs": folded["signs"].astype(ml_dtypes.bfloat16),
            "ones2": np.ones((1, CFG2["group"] * 512), dtype=np.float32),
            "x": np.ascontiguousarray(x0[BC * c:BC * (c + 1)]),
        }
        in_maps.append(m)
    res = run_bass_kernel_spmd(nc, in_maps, core_ids=list(range(NCORES)),
                               trace=trace, **kw)
    out = np.concatenate(
        [res.results[c]["out"].reshape(BC, 1) for c in range(NCORES)], axis=0
    )
    return out, res


def _run(inputs, trace=False, **kw):
    folded, bconst = fold_weights(
        inputs["W1"], inputs["b1"], inputs["W2"], inputs["b2"],
        inputs["W3"], inputs["b3"], inputs["W_out"], inputs["b_out"],
    )
    nc = build_module(bconst)
    x0 = np.ascontiguousarray(np.asarray(inputs["x0"], dtype=np.float32))
    in_maps = []
    for c in range(NCORES):
        m = dict(folded)
        m["x"] = np.ascontiguousarray(x0[BC * c:BC * (c + 1)])
        in_maps.append(m)
    res = run_bass_kernel_spmd(nc, in_maps, core_ids=list(range(NCORES)),
                               trace=trace, **kw)
    out = np.concatenate(
        [res.results[c]["out"].reshape(BC, 1) for c in range(NCORES)], axis=0
    )
    return out, res


def kernel(**inputs) -> np.ndarray:
    out, _ = _run7(inputs, trace=False)
    return out



# revision 69
# speedup vs baseline: 1.0630x; 1.0630x over previous
"""Trainium2 Bass kernel for nn_CompressedInteractionNetwork_9105330667837.

Algorithm: the network output is (B,1) only, so the 3-layer CIN collapses
algebraically to a per-(b,d)-column quartic form evaluated as
    out[b] = B_const + sum_d [ g(x).t(x) + x.u(x) ],   x = x0[b,:,d] in R^32
with g[o] = x^T W1[o] x (64 quadratic forms), t[k] = x^T U3[k] x + V2[k].x,
u = Asym x + s23.  All quadratic forms are evaluated through a shared
"squares basis": z = LIN @ x (pair-sums), basis = [z^2; x^2; x_m x_{m+16}; x],
then [g;t] = R @ basis.  Everything contracts on the PE in float32r; squares
on ScalarE; products/reduction on VectorE/GpSimd.

Sharding: data-parallel over batch across 8 cores (weights replicated).
"""

import numpy as np
from contextlib import ExitStack

import concourse.bass as bass
from concourse import bacc
import concourse.mybir as mybir
import concourse.tile as tile
from concourse.bass_utils import run_bass_kernel_spmd
from concourse import dve_ops as _dvo
from concourse.dve_spec import Spec as _Spec, Src0 as _Src0, Bin as _Bin, AluOp as _AluOp
from concourse.dve_table_gen import dve_ver_for as _dve_ver_for


def _register_square_op():
    if "SQUARE_ANT" in _dvo._SUB_OPCODE_FOR_NAME:
        return _dvo.CUSTOM_DVE_SPECS and [op for op in _dvo.OPS if op.name == "SQUARE_ANT"][0]
    op = _dvo.DveOp(
        "SQUARE_ANT",
        _Spec(
            body=_Bin(_AluOp.MULTIPLY, _Src0, _Src0),
            reference=lambda in0, in1, s0, s1, imm2: (
                in0.astype(np.float32) * in0.astype(np.float32)
            ),
        ),
        subdim=False,
        uops_sha={},
    )
    _dvo.OPS.append(op)
    _dvo.CUSTOM_DVE_SPECS[op.name] = op.spec
    _dvo._SUB_OPCODE_FOR_NAME[op.name] = max(_dvo._SUB_OPCODE_FOR_NAME.values()) + 1
    for ver in ("v3", "v4"):
        try:
            op.compile(ver)
        except ValueError as e:
            import re as _re
            m = _re.search(r": ([0-9a-f]{16}) ", str(e))
            if m is None:
                raise
            op.uops_sha[ver] = m.group(1)
            _dvo._COMPILE_CACHE.pop((op.name, ver), None)
            op.compile(ver)
    return op


SQUARE_ANT = _register_square_op()


def _patch_dve_root_path():
    """walrus requires --dve-root-json absolute; the bass2jax hook path can
    hand it a relative path. Resolve against plausible bases."""
    import os
    import concourse.bass_utils as bu

    if getattr(bu, "_dve_path_patched", False):
        return
    orig = bu.run_command

    def patched(cmd, cwd=None, **kw):
        try:
            i = cmd.index("--dve-root-json") + 1
            p = cmd[i]
            if not os.path.isabs(p):
                for base in (cwd or ".", os.path.join(cwd or ".", ".."), "."):
                    cand = os.path.abspath(os.path.join(base, p))
                    if os.path.exists(cand):
                        cmd[i] = cand
                        break
        except ValueError:
            pass
        return orig(cmd, cwd=cwd, **kw)

    bu.run_command = patched
    bu._dve_path_patched = True


_patch_dve_root_path()

B, F, D = 2048, 32, 64
NCORES = 8
BC = B // NCORES            # 256 batches per core
CHUNK_B = 8                 # batches per chunk
P = CHUNK_B * D             # 512 pairs per chunk
NCHUNK = BC // CHUNK_B      # 32

SPECIAL = [(m, m + 16) for m in range(16)]          # pairs done as direct products
_SP = set(SPECIAL)
PAIRS = [(a, b) for a in range(F) for b in range(a + 1, F) if (a, b) not in _SP]
assert len(PAIRS) == 480

f32 = mybir.dt.float32
f32r = mybir.dt.float32r


def fold_weights(W1, b1, W2, b2, W3, b3, W_out, b_out):
    """Host-side folding. Returns dict of small fp32 arrays + bconst float."""
    W1, b1, W2, b2, W3, b3, W_out, b_out = [
        np.asarray(a, dtype=np.float64) for a in (W1, b1, W2, b2, W3, b3, W_out, b_out)
    ]
    w1, w2, w3 = W_out[0:64, 0], W_out[64:128, 0], W_out[128:192, 0]

    V2 = np.einsum("o,ohm->hm", w2, W2)           # (64,32)
    V3 = np.einsum("o,ohm->hm", w3, W3)           # (64,32)
    U3 = np.einsum("hkm,hn->kmn", W2, V3)         # (64,32,32)
    V1 = np.einsum("o,ohm->hm", w1, W1)           # (32,32)
    Le = np.einsum("k,kmn->mn", b1, U3)           # (32,32)
    A = V1 + Le
    Asym = (A + A.T) / 2
    s23 = V2.T @ b1 + V3.T @ b2                   # (32,)
    bconst = D * (w1 @ b1 + w2 @ b2 + w3 @ b3) + b_out[0]

    M1s = (W1 + W1.transpose(0, 2, 1)) / 2        # 64 sym forms for g
    U3s = (U3 + U3.transpose(0, 2, 1)) / 2        # 64 sym forms for t

    # LIN lhsT: (32, 4*128). Tile j rows: j<3 -> PAIRS[128j:128j+128] sums;
    # tile 3 -> PAIRS[384:480] sums (96 rows) + Asym rows (32).
    LINW = np.zeros((F, 4 * 128))
    for j in range(4):
        rows = PAIRS[128 * j: 128 * (j + 1)]
        for i, (a, b_) in enumerate(rows):
            LINW[a, 128 * j + i] += 1.0
            LINW[b_, 128 * j + i] += 1.0
        if j == 3:
            LINW[:, 128 * 3 + 96: 128 * 3 + 128] = Asym.T  # rows 96..127 = Asym @ x

    # Big-matmul lhsT per chain: RW (128, 5*128): RW[k, 128j+? ...] wait layout:
    # lhsT for chain j is (K_j, 128): RW[0:K_j, j-block], K_j = 128 (j<4) or 48.
    # outputs: m<64 -> form M1s[m], v=0 ; m>=64 -> form U3s[m-64], v=V2[m-64]
    forms = np.concatenate([M1s, U3s], axis=0)    # (128, 32, 32)
    linv = np.concatenate([np.zeros((64, F)), V2], axis=0)  # (128, 32)

    # rw layout: rw[k, 128*j + m] = weight of chain-j basis-row k for output m
    RW = np.zeros((128, 5 * 128))
    # chains 0-2: squares of pair-sums
    for j in range(3):
        rows = PAIRS[128 * j: 128 * (j + 1)]
        for i, (a, b_) in enumerate(rows):
            RW[i, 128 * j:128 * (j + 1)] = forms[:, a, b_]
    # chain 3: rows 0-95 squares of PAIRS[384:480]; rows 96-127 x^2
    for i, (a, b_) in enumerate(PAIRS[384:480]):
        RW[i, 128 * 3:128 * 4] = forms[:, a, b_]
    # x^2 weights: S[m,m] - sum_{(a,b) in PAIRS containing m} S[a,b]
    corr = np.zeros((128, F))
    for (a, b_) in PAIRS:
        corr[:, a] += forms[:, a, b_]
        corr[:, b_] += forms[:, a, b_]
    # chain 4 (K=80): rows 0-31 x^2; 32-63 x; 64-79 products x_m x_{m+16}
    for i, (a, b_) in enumerate(SPECIAL):
        RW[64 + i, 128 * 4:128 * 5] = 2.0 * forms[:, a, b_]
    for m in range(F):
        RW[32 + m, 128 * 4:128 * 5] = linv[:, m]
        RW[m, 128 * 4:128 * 5] = forms[:, m, m] - corr[:, m]

    return {
        "linw": LINW.astype(np.float32),
        "rw": RW.astype(np.float32),
        "s23": s23.reshape(F, 1).astype(np.float32),
        "ones": np.ones((96, 1), dtype=np.float32),
    }, float(bconst)


PAIRS2 = [(a, b) for a in range(F) for b in range(a + 1, F)]   # all 496
assert len(PAIRS2) == 496
NV = 577            # v' = [z2(512, 16 spare); x2(32); x(32); const(1)]
IX_X2 = 512
IX_X = 544
IX_C = 576


def _form_to_v(S):
    """Coefficient vector c in R^577 with c.v' = x^T S x (S sym, 32x32)."""
    c = np.zeros(NV)
    for i, (a, b) in enumerate(PAIRS2):
        j, k = divmod(i, 128)
        c[128 * j + k] = S[a, b]
    for m in range(F):
        c[IX_X2 + m] = S[m, m] - sum(
            S[a, b] for (a, b) in PAIRS2 if a == m or b == m
        )
    return c


def fold_weights2(W1, b1, W2, b2, W3, b3, W_out, b_out):
    """v2 folding: quartic = sum_i s_i * (rw2 @ v')_i^2 + bconst, where
    v' = [pair-sum squares(496+16 spare); x^2(32); x(32); 1]."""
    W1, b1, W2, b2, W3, b3, W_out, b_out = [
        np.asarray(a, dtype=np.float64) for a in (W1, b1, W2, b2, W3, b3, W_out, b_out)
    ]
    w1, w2, w3 = W_out[0:64, 0], W_out[64:128, 0], W_out[128:192, 0]
    V2 = np.einsum("o,ohm->hm", w2, W2)           # (64,32)
    V3 = np.einsum("o,ohm->hm", w3, W3)           # (64,32)
    U3 = np.einsum("hkm,hn->kmn", W2, V3)         # (64,32,32)

    M1s = (W1 + W1.transpose(0, 2, 1)) / 2
    U3s = (U3 + U3.transpose(0, 2, 1)) / 2

    # Bias-folded pairing: total = sum_k ghat_k that_k + (V3^T b2).x + bconst
    # with ghat_k = x^T M1s[k] x + b1_k, that_k = x^T U3s[k] x + V2[k].x + w1_k
    # and bconst = D*(w2.b2 + w3.b3) + b_out.
    G = np.stack([_form_to_v(M1s[k]) for k in range(64)])      # (64, 577)
    T = np.stack([_form_to_v(U3s[k]) for k in range(64)])      # (64, 577)
    for k in range(64):
        G[k, IX_C] = b1[k]
        T[k, IX_X:IX_X + F] += V2[k]
        T[k, IX_C] = w1[k]
    ell = np.zeros(NV)
    ell[IX_X:IX_X + F] = V3.T @ b2
    bconst = D * (w2 @ b2 + w3 @ b3) + b_out[0]

    C = G.T @ T
    C[IX_C, :] += ell
    Cs = (C + C.T) / 2
    lam, vec = np.linalg.eigh(Cs)
    order = np.argsort(-np.abs(lam))
    keep = order[:128]
    trunc = float(np.abs(lam[order[128:]]).sum())
    lead = float(np.abs(lam[keep]).sum())
    rw2 = (vec[:, keep] * np.sqrt(np.abs(lam[keep]))).T        # (128, 577)
    signs = np.sign(lam[keep])

    # LIN lhsT: (32*4 partitions, 4*128 cols): chain j rows 32j..32j+32,
    # cols 128j..128(j+1); pair-sum weights.
    LIN2 = np.zeros((128, 4 * 128))
    for i, (a, b) in enumerate(PAIRS2):
        j, k = divmod(i, 128)
        LIN2[32 * j + a, 128 * j + k] = 1.0
        LIN2[32 * j + b, 128 * j + k] = 1.0

    # big-matmul lhsT blocks: chains j<4: (128,128) = rw2 cols 128j..;
    # chain 4: (65, 128) = rw2 cols [x2, x, const]
    RWB = np.zeros((128, 5 * 128))
    for j in range(4):
        RWB[:, 128 * j:128 * (j + 1)] = rw2[:, 128 * j:128 * (j + 1)].T
    RWB[0:65, 512:640] = rw2[:, 512:577].T

    return {
        "linw2": LIN2.astype(np.float32),
        "rwb": RWB.astype(np.float32),
        "signs": signs.reshape(128, 1).astype(np.float32),
    }, float(bconst), {"trunc": trunc, "lead": lead}


_module_cache = {}


CFG = {"P": 512, "lin_split": 2, "lin_bufs": 2, "big_bufs": 2, "gp_d16": True,
       "dve_sq": False,
       "no_sq": False, "no_gs": False, "no_dve": False, "no_gp": False,
       "no_lin": False, "no_big": False, "no_dma": False}


def build_module(bconst: float, repeat: int = 1):
    key = (round(bconst, 12), repeat, tuple(sorted(CFG.items())))
    if key in _module_cache:
        return _module_cache[key]
    nc = bacc.Bacc("TRN2", target_bir_lowering=False)
    x_d = nc.dram_tensor("x", [BC, F, D], f32r, kind="ExternalInput")
    linw_d = nc.dram_tensor("linw", [F, 4 * 128], f32r, kind="ExternalInput")
    rw_d = nc.dram_tensor("rw", [128, 5 * 128], f32r, kind="ExternalInput")
    s23_d = nc.dram_tensor("s23", [F, 1], f32, kind="ExternalInput")
    ones_d = nc.dram_tensor("ones", [96, 1], f32r, kind="ExternalInput")
    out_d = nc.dram_tensor("out", [1, BC], f32, kind="ExternalOutput")

    SQ = mybir.ActivationFunctionType.Square
    CP = mybir.ActivationFunctionType.Copy
    ADD = mybir.AluOpType.add
    MULT = mybir.AluOpType.mult

    with tile.TileContext(nc) as tc, ExitStack() as ctx:
        const = ctx.enter_context(tc.tile_pool(name="const", bufs=1))
        xp = ctx.enter_context(tc.tile_pool(name="xp", bufs=3))
        chp = ctx.enter_context(tc.tile_pool(name="chp", bufs=10))
        ch4p = ctx.enter_context(tc.tile_pool(name="ch4p", bufs=3))
        prp = ctx.enter_context(tc.tile_pool(name="prp", bufs=1))
        gsp = ctx.enter_context(tc.tile_pool(name="gsp", bufs=3))
        outp = ctx.enter_context(tc.tile_pool(name="outp", bufs=1))
        linps = ctx.enter_context(
            tc.tile_pool(name="linps", bufs=CFG["lin_bufs"], space="PSUM"))
        bigps = ctx.enter_context(tc.tile_pool(name="bigps", bufs=CFG["big_bufs"], space="PSUM"))
        finps = ctx.enter_context(tc.tile_pool(name="finps", bufs=1, space="PSUM"))

        linw_t = const.tile([F, 4 * 128], f32r)
        nc.sync.dma_start(linw_t[:], linw_d[:])
        rw_t = const.tile([128, 5 * 128], f32r)
        nc.sync.dma_start(rw_t[:], rw_d[:])
        s23_t = const.tile([F, 1], f32)
        nc.sync.dma_start(s23_t[:], s23_d[:])
        ones_t = const.tile([96, 1], f32r)
        nc.sync.dma_start(ones_t[:], ones_d[:])

        pr_all = prp.tile([96, BC * D], f32r)

        rep_ctx = tc.For_i(0, repeat, 1) if repeat > 1 else None
        if rep_ctx is not None:
            ctx.enter_context(rep_ctx)

        CP_ = CFG["P"]          # pairs per chunk
        CB = CP_ // D           # batches per chunk
        NCH = BC // CB          # chunks
        NT = CP_ // 512         # matmul N-tiles per chunk
        GROUP = max(1, 2048 // CP_)
        for g in range(NCH // GROUP):
            b0 = g * GROUP * CB
            nb = GROUP * CB
            xsrc = x_d[b0:b0 + nb].transpose([1, 0, 2])   # (32, nb, 64)
            xg_t = xp.tile([F, GROUP * CP_], f32r, tag="x")
            nc.sync.dma_start(
                xg_t[:].rearrange("k (b d) -> k b d", b=nb), xsrc
            )
            ch4g = ch4p.tile([80, GROUP * CP_], f32r, tag="ch4")
            nc.sync.dma_start(
                ch4g[32:64].rearrange("k (b d) -> k b d", b=nb), xsrc
            )
            xs_g = xp.tile([16, GROUP * CP_], f32r, tag="xs")
            nc.sync.dma_start(
                xs_g[:].rearrange("k (b d) -> k b d", b=nb),
                x_d[b0:b0 + nb, 16:32, :].transpose([1, 0, 2]),
            )
            for ci in range(GROUP):
                cs = slice(ci * CP_, (ci + 1) * CP_)
                x_t = xg_t[:, cs]
                ch4 = ch4g[:, cs]

                ns = CFG["lin_split"]  # chains per lin psum tile
                ntile = 4 // ns
                lptiles = []
                chains = []
                for t in range(ntile):
                    lp = linps.tile([128, ns * CP_], f32, tag="lp")
                    for jj in range(ns):
                        j = t * ns + jj
                        for nt in range(NT):
                            nc.tensor.matmul(
                                lp[:, jj * CP_ + nt * 512:jj * CP_ + (nt + 1) * 512],
                                linw_t[:, 128 * j:128 * (j + 1)],
                                x_t[:, nt * 512:(nt + 1) * 512],
                                start=True, stop=True,
                            )
                    lptiles.append(lp)
                    chn = chp.tile([128, ns * CP_], f32r, tag="ch")
                    c_ = g * GROUP + ci
                    if t == ntile - 1 and ntile > 1 and c_ % 2 == 1 and CFG["dve_sq"]:
                        nc.vector._custom_dve(SQUARE_ANT, out=chn[:], in0=lp[:])
                    else:
                        nc.scalar.activation(chn[:], lp[:], SQ)
                    for jj in range(ns):
                        chains.append(chn[:, jj * CP_:(jj + 1) * CP_])
                lp_u = lptiles[-1]
                nc.gpsimd.tensor_mul(ch4[0:32], x_t, x_t)
                if CFG["gp_d16"]:
                    nc.gpsimd.tensor_mul(ch4[64:80], x_t[0:16], xs_g[:, cs])
                else:
                    nc.vector.tensor_mul(ch4[64:80], x_t[0:16], xs_g[:, cs])
                chains.append(ch4)

                bp = bigps.tile([128, CP_], f32, tag="bp")
                for j in range(5):
                    K_j = 128 if j < 4 else 80
                    for nt in range(NT):
                        nc.tensor.matmul(
                            bp[:, nt * 512:(nt + 1) * 512],
                            rw_t[0:K_j, 128 * j:128 * (j + 1)],
                            chains[j][0:K_j, nt * 512:(nt + 1) * 512],
                            start=(j == 0), stop=(j == 4),
                        )

                c = g * GROUP + ci
                pcs = slice(c * CP_, (c + 1) * CP_)
                gs = gsp.tile([64, CP_], f32, tag="gs")
                nc.scalar.activation(gs[:], bp[0:64], CP)
                nc.vector.tensor_mul(pr_all[0:64, pcs], gs[:], bp[64:128])
                nc.vector.scalar_tensor_tensor(
                    pr_all[64:96, pcs], lp_u[96:128, (ns - 1) * CP_:ns * CP_],
                    s23_t[:], x_t, ADD, MULT
                )

        fp = finps.tile([1, BC], f32)
        pr3 = pr_all[:].rearrange("p (b d) -> p b d", b=BC)
        for d in range(D):
            nc.tensor.matmul(
                fp[:], ones_t[:], pr3[:, :, d],
                start=(d == 0), stop=(d == D - 1),
            )
        out_sb = outp.tile([1, BC], f32)
        nc.scalar.activation(out_sb[:], fp[:], CP, bias=float(bconst))
        nc.sync.dma_start(out_d[:], out_sb[:])

    nc.compile()
    _module_cache[key] = nc
    return nc


bf16 = mybir.dt.bfloat16

# v2 config: engine assignment for the 4 z^2 ops, x^2 op, y^2 op.
# engines: "s" = scalar (activation Square), "v" = vector (custom dve square),
# "g" = gpsimd (tensor_mul; SBUF operands only!)
CFG2 = {
    "sq_eng": ["s", "v", "s", "v"],   # per LIN chain z^2
    "x2_eng": "g",
    "y2_eng": "s",
    "group": 4,
}


def build_module2(bconst: float, repeat: int = 1):
    key = ("v2", round(bconst, 12), repeat,
           tuple(CFG2["sq_eng"]), CFG2["x2_eng"], CFG2["y2_eng"], CFG2["group"])
    if key in _module_cache:
        return _module_cache[key]
    nc = bacc.Bacc("TRN2", target_bir_lowering=False)
    x_d = nc.dram_tensor("x", [BC, F, D], f32r, kind="ExternalInput")
    linw_d = nc.dram_tensor("linw2", [128, 4 * 128], f32r, kind="ExternalInput")
    rwb_d = nc.dram_tensor("rwb", [128, 5 * 128], f32r, kind="ExternalInput")
    signs_d = nc.dram_tensor("signs", [128, 1], bf16, kind="ExternalInput")
    ones_d = nc.dram_tensor("ones2", [1, CFG2["group"] * 512], f32r,
                            kind="ExternalInput")
    out_d = nc.dram_tensor("out", [1, BC], f32, kind="ExternalOutput")

    SQ = mybir.ActivationFunctionType.Square
    CP_ = 512                  # pair-cols per chunk
    CB = CP_ // D              # 8 batches per chunk
    NCH = BC // CB             # 32 chunks
    GROUP = CFG2["group"]      # chunks per DMA group
    QCH = 8                    # chunks per tail quarter (64 batches)

    def sq_op(eng, out, in_):
        if eng == "s":
            nc.scalar.activation(out, in_, SQ)
        elif eng == "v":
            nc.vector._custom_dve(SQUARE_ANT, out=out, in0=in_)
        else:
            nc.gpsimd.tensor_mul(out, in_, in_)

    with tile.TileContext(nc) as tc, ExitStack() as ctx:
        const = ctx.enter_context(tc.tile_pool(name="const", bufs=1))
        x4p = ctx.enter_context(tc.tile_pool(name="x4p", bufs=2))
        ch4p = ctx.enter_context(tc.tile_pool(name="ch4p", bufs=2))
        chp = ctx.enter_context(tc.tile_pool(name="chp", bufs=4))
        prp = ctx.enter_context(tc.tile_pool(name="prp", bufs=2))
        outp = ctx.enter_context(tc.tile_pool(name="outp", bufs=1))
        zpsA = ctx.enter_context(tc.tile_pool(name="zpsA", bufs=1, space="PSUM"))
        zpsB = ctx.enter_context(tc.tile_pool(name="zpsB", bufs=1, space="PSUM"))
        yps = ctx.enter_context(tc.tile_pool(name="yps", bufs=2, space="PSUM"))
        qps = ctx.enter_context(tc.tile_pool(name="qps", bufs=2, space="PSUM"))

        linw_t = const.tile([128, 4 * 128], f32r)
        nc.sync.dma_start(linw_t[:], linw_d[:])
        rwb_t = const.tile([128, 5 * 128], f32r)
        nc.sync.dma_start(rwb_t[:], rwb_d[:])
        signs_t = const.tile([128, 1], bf16)
        nc.sync.dma_start(signs_t[:], signs_d[:])
        out_acc = outp.tile([1, BC], f32)
        out_fin = outp.tile([1, BC], f32)

        rep_ctx = tc.For_i(0, repeat, 1) if repeat > 1 else None
        if rep_ctx is not None:
            ctx.enter_context(rep_ctx)

        def stage_a(x4_t, ch4g, cs):
            zA = zpsA.tile([128, 2 * CP_], f32, tag="zA")
            zB = zpsB.tile([128, 2 * CP_], f32, tag="zB")
            for j, (zt, off) in enumerate(
                    [(zA, 0), (zA, CP_), (zB, 0), (zB, CP_)]):
                nc.tensor.matmul(
                    zt[:, off:off + CP_],
                    linw_t[32 * j:32 * (j + 1), 128 * j:128 * (j + 1)],
                    x4_t[32 * j:32 * (j + 1), cs],
                    start=True, stop=True,
                    tile_position=(32 * j, 0),
                )
            chA = chp.tile([128, 2 * CP_], f32r, tag="chA")
            chB = chp.tile([128, 2 * CP_], f32r, tag="chB")
            nc.scalar.activation(chA[:], zA[:], SQ)
            nc.vector._custom_dve(SQUARE_ANT, out=chB[:], in0=zB[:])
            nc.gpsimd.tensor_mul(ch4g[0:32, cs], x4_t[0:32, cs], x4_t[0:32, cs])
            return (chA, chB, ch4g, cs)

        def stage_b(st, pr_t, pc):
            chA, chB, ch4g, cs = st
            yt = yps.tile([128, CP_], f32, tag="y")
            rhs = [chA[:, 0:CP_], chA[:, CP_:2 * CP_],
                   chB[:, 0:CP_], chB[:, CP_:2 * CP_]]
            for j in range(4):
                nc.tensor.matmul(yt[:], rwb_t[:, 128 * j:128 * (j + 1)],
                                 rhs[j], start=(j == 0), stop=False)
            nc.tensor.matmul(yt[:], rwb_t[0:65, 512:640], ch4g[0:65, cs],
                             start=False, stop=True)
            h = CP_ // 2
            nc.scalar.activation(pr_t[:, pc:pc + h], yt[:, 0:h], SQ)
            nc.vector._custom_dve(SQUARE_ANT, out=pr_t[:, pc + h:pc + CP_],
                                  in0=yt[:, h:CP_])

        NQ = NCH // QCH
        for q in range(NQ):                    # tail quarters (64 batches)
            pr_t = prp.tile([128, QCH * CP_], bf16, tag="pr")
            pend = None
            for gg in range(QCH // GROUP):     # DMA groups within quarter
                g = q * (QCH // GROUP) + gg
                b0 = g * GROUP * CB
                nb = GROUP * CB
                xsrc = x_d[b0:b0 + nb].transpose([1, 0, 2])   # (32, nb, 64)
                x4_t = x4p.tile([128, GROUP * CP_], f32r, tag="x4")
                for j in range(4):
                    nc.sync.dma_start(
                        x4_t[32 * j:32 * (j + 1)].rearrange(
                            "k (b d) -> k b d", b=nb), xsrc)
                ch4g = ch4p.tile([65, GROUP * CP_], f32r, tag="ch4")
                nc.sync.dma_start(
                    ch4g[32:64].rearrange("k (b d) -> k b d", b=nb), xsrc)
                if g < 2:
                    nc.sync.dma_start(ch4g[64:65], ones_d[:])

                for ci in range(GROUP):
                    cs = slice(ci * CP_, (ci + 1) * CP_)
                    c_in_q = gg * GROUP + ci
                    st = stage_a(x4_t, ch4g, cs)
                    if pend is not None:
                        stage_b(*pend)
                    pend = (st, pr_t, c_in_q * CP_)
            stage_b(*pend)
            # tail: 8 accumulating N=512 passes (8-d blocks), then a
            # segmented 8->1 reduce on VectorE into out_acc.
            qacc = qps.tile([1, CP_], f32, tag="qacc")
            pr4 = pr_t[:].rearrange("p (b d) -> p b d", d=D)
            for dd in range(8):
                nc.tensor.matmul(
                    qacc[:], signs_t[:], pr4[:, :, 8 * dd:8 * (dd + 1)],
                    start=(dd == 0), stop=(dd == 7),
                )
            nc.vector.tensor_reduce(
                out_acc[0:1, q * QCH * CB:(q + 1) * QCH * CB],
                qacc[:].rearrange("p (b d) -> p b d", d=8),
                mybir.AxisListType.X, mybir.AluOpType.add,
            )
        nc.scalar.activation(out_fin[:], out_acc[:],
                             mybir.ActivationFunctionType.Copy,
                             bias=float(bconst))
        nc.sync.dma_start(out_d[:], out_fin[:])

    nc.compile()
    _module_cache[key] = nc
    return nc


def build_module3(bconst: float, repeat: int = 1):
    """v3: same math as v2, restructured for back-to-back PE issue.

    - 4 LIN chains write 4 separate single-bank PSUM tiles (pool depth 5)
      so the next chunk's LIN matmuls never wait on this chunk's squares.
    - squares: 2 on ScalarE + 2 on VectorE per chunk; yt square split as in v2.
    """
    key = ("v3", round(bconst, 12), repeat)
    if key in _module_cache:
        return _module_cache[key]
    nc = bacc.Bacc("TRN2", target_bir_lowering=False)
    x_d = nc.dram_tensor("x", [BC, F, D], f32r, kind="ExternalInput")
    linw_d = nc.dram_tensor("linw2", [128, 4 * 128], f32r, kind="ExternalInput")
    rwb_d = nc.dram_tensor("rwb", [128, 5 * 128], f32r, kind="ExternalInput")
    signs_d = nc.dram_tensor("signs", [128, 1], bf16, kind="ExternalInput")
    ones_d = nc.dram_tensor("ones2", [1, 4 * 512], f32r, kind="ExternalInput")
    out_d = nc.dram_tensor("out", [1, BC], f32, kind="ExternalOutput")

    SQ = mybir.ActivationFunctionType.Square
    CP_ = 512                  # pair-cols per chunk
    CB = CP_ // D              # 8 batches per chunk
    NCH = BC // CB             # 32 chunks
    GROUP = 4                  # chunks per DMA group
    QCH = 8                    # chunks per tail quarter (64 batches)

    with tile.TileContext(nc) as tc, ExitStack() as ctx:
        const = ctx.enter_context(tc.tile_pool(name="const", bufs=1))
        x4p = ctx.enter_context(tc.tile_pool(name="x4p", bufs=2))
        ch4p = ctx.enter_context(tc.tile_pool(name="ch4p", bufs=2))
        chp = ctx.enter_context(tc.tile_pool(name="chp", bufs=8))
        prp = ctx.enter_context(tc.tile_pool(name="prp", bufs=2))
        outp = ctx.enter_context(tc.tile_pool(name="outp", bufs=1))
        zps = ctx.enter_context(tc.tile_pool(name="zps", bufs=5, space="PSUM"))
        yps = ctx.enter_context(tc.tile_pool(name="yps", bufs=2, space="PSUM"))
        qps = ctx.enter_context(tc.tile_pool(name="qps", bufs=1, space="PSUM"))

        linw_t = const.tile([128, 4 * 128], f32r)
        nc.sync.dma_start(linw_t[:], linw_d[:])
        rwb_t = const.tile([128, 5 * 128], f32r)
        nc.sync.dma_start(rwb_t[:], rwb_d[:])
        signs_t = const.tile([128, 1], bf16)
        nc.sync.dma_start(signs_t[:], signs_d[:])
        out_acc = outp.tile([1, BC], f32)
        out_fin = outp.tile([1, BC], f32)

        rep_ctx = tc.For_i(0, repeat, 1) if repeat > 1 else None
        if rep_ctx is not None:
            ctx.enter_context(rep_ctx)

        def stage_a(x4_t, ch4g, cs):
            chs = []
            for j in range(4):
                zt = zps.tile([128, CP_], f32, tag="z")
                nc.tensor.matmul(
                    zt[:],
                    linw_t[32 * j:32 * (j + 1), 128 * j:128 * (j + 1)],
                    x4_t[32 * j:32 * (j + 1), cs],
                    start=True, stop=True,
                    tile_position=(32 * j, 0),
                )
                ch = chp.tile([128, CP_], f32r, tag="ch")
                if j % 2 == 0:
                    nc.scalar.activation(ch[:], zt[:], SQ)
                else:
                    nc.vector._custom_dve(SQUARE_ANT, out=ch[:], in0=zt[:])
                chs.append(ch)
            nc.gpsimd.tensor_mul(ch4g[0:32, cs], x4_t[0:32, cs], x4_t[0:32, cs])
            return (chs, ch4g, cs)

        def stage_b(st, pr_t, pc):
            chs, ch4g, cs = st
            yt = yps.tile([128, CP_], f32, tag="y")
            for j in range(4):
                nc.tensor.matmul(yt[:], rwb_t[:, 128 * j:128 * (j + 1)],
                                 chs[j][:], start=(j == 0), stop=False)
            nc.tensor.matmul(yt[:], rwb_t[0:65, 512:640], ch4g[0:65, cs],
                             start=False, stop=True)
            h = CP_ // 2
            nc.scalar.activation(pr_t[:, pc:pc + h], yt[:, 0:h], SQ)
            nc.vector._custom_dve(SQUARE_ANT, out=pr_t[:, pc + h:pc + CP_],
                                  in0=yt[:, h:CP_])

        NQ = NCH // QCH
        for q in range(NQ):                    # tail quarters (64 batches)
            pr_t = prp.tile([128, QCH * CP_], bf16, tag="pr")
            pend = None
            for gg in range(QCH // GROUP):     # DMA groups within quarter
                g = q * (QCH // GROUP) + gg
                b0 = g * GROUP * CB
                nb = GROUP * CB
                xsrc = x_d[b0:b0 + nb].transpose([1, 0, 2])   # (32, nb, 64)
                x4_t = x4p.tile([128, GROUP * CP_], f32r, tag="x4")
                for j in range(4):
                    nc.sync.dma_start(
                        x4_t[32 * j:32 * (j + 1)].rearrange(
                            "k (b d) -> k b d", b=nb), xsrc)
                ch4g = ch4p.tile([65, GROUP * CP_], f32r, tag="ch4")
                nc.sync.dma_start(
                    ch4g[32:64].rearrange("k (b d) -> k b d", b=nb), xsrc)
                if g < 2:
                    nc.sync.dma_start(ch4g[64:65], ones_d[:])

                for ci in range(GROUP):
                    cs = slice(ci * CP_, (ci + 1) * CP_)
                    c_in_q = gg * GROUP + ci
                    st = stage_a(x4_t, ch4g, cs)
                    if pend is not None:
                        stage_b(*pend)
                    pend = (st, pr_t, c_in_q * CP_)
            stage_b(*pend)
            # tail: 8 accumulating N=512 passes (8-d blocks), then a
            # segmented 8->1 reduce on VectorE into out_acc.
            qacc = qps.tile([1, CP_], f32, tag="qacc")
            pr4 = pr_t[:].rearrange("p (b d) -> p b d", d=D)
            for dd in range(8):
                nc.tensor.matmul(
                    qacc[:], signs_t[:], pr4[:, :, 8 * dd:8 * (dd + 1)],
                    start=(dd == 0), stop=(dd == 7),
                )
            nc.vector.tensor_reduce(
                out_acc[0:1, q * QCH * CB:(q + 1) * QCH * CB],
                qacc[:].rearrange("p (b d) -> p b d", d=8),
                mybir.AxisListType.X, mybir.AluOpType.add,
            )
        nc.scalar.activation(out_fin[:], out_acc[:],
                             mybir.ActivationFunctionType.Copy,
                             bias=float(bconst))
        nc.sync.dma_start(out_d[:], out_fin[:])

    nc.compile()
    _module_cache[key] = nc
    return nc


def build_module4(bconst: float, repeat: int = 1):
    """v4: bf16 z-drain + bf16 big-matmul chains.

    - LIN matmuls drain to bf16 PSUM tiles ([128,1024] = 1 bank for a
      2-chain pair) -> halved PSUM pressure, 16-bit DVE squares.
    - big matmul chains 0-3 in bf16 (FWL weight loads); chain 4 (x^2,x,1)
      and yt accumulation stay f32 for accuracy.
    """
    key = ("v4", round(bconst, 12), repeat)
    if key in _module_cache:
        return _module_cache[key]
    nc = bacc.Bacc("TRN2", target_bir_lowering=False)
    x_d = nc.dram_tensor("x", [BC, F, D], f32r, kind="ExternalInput")
    linw_d = nc.dram_tensor("linw2", [128, 4 * 128], f32r, kind="ExternalInput")
    rwb03_d = nc.dram_tensor("rwb03", [128, 4 * 128], bf16, kind="ExternalInput")
    rwb4_d = nc.dram_tensor("rwb4", [65, 128], f32r, kind="ExternalInput")
    signs_d = nc.dram_tensor("signs", [128, 1], bf16, kind="ExternalInput")
    ones_d = nc.dram_tensor("ones2", [1, 4 * 512], f32r, kind="ExternalInput")
    out_d = nc.dram_tensor("out", [1, BC], f32, kind="ExternalOutput")

    SQ = mybir.ActivationFunctionType.Square
    CP_ = 512                  # pair-cols per chunk
    CB = CP_ // D              # 8 batches per chunk
    NCH = BC // CB             # 32 chunks
    GROUP = 4                  # chunks per DMA group
    QCH = 8                    # chunks per tail quarter (64 batches)

    with tile.TileContext(nc) as tc, ExitStack() as ctx:
        const = ctx.enter_context(tc.tile_pool(name="const", bufs=1))
        x4p = ctx.enter_context(tc.tile_pool(name="x4p", bufs=2))
        ch4p = ctx.enter_context(tc.tile_pool(name="ch4p", bufs=2))
        chp = ctx.enter_context(tc.tile_pool(name="chp", bufs=4))
        prp = ctx.enter_context(tc.tile_pool(name="prp", bufs=2))
        outp = ctx.enter_context(tc.tile_pool(name="outp", bufs=1))
        zpsA = ctx.enter_context(tc.tile_pool(name="zpsA", bufs=1, space="PSUM"))
        zpsB = ctx.enter_context(tc.tile_pool(name="zpsB", bufs=1, space="PSUM"))
        yps = ctx.enter_context(tc.tile_pool(name="yps", bufs=2, space="PSUM"))
        qps = ctx.enter_context(tc.tile_pool(name="qps", bufs=2, space="PSUM"))

        linw_t = const.tile([128, 4 * 128], f32r)
        nc.sync.dma_start(linw_t[:], linw_d[:])
        rwb03_t = const.tile([128, 4 * 128], bf16)
        nc.sync.dma_start(rwb03_t[:], rwb03_d[:])
        rwb4_t = const.tile([65, 128], f32r)
        nc.sync.dma_start(rwb4_t[:], rwb4_d[:])
        signs_t = const.tile([128, 1], bf16)
        nc.sync.dma_start(signs_t[:], signs_d[:])
        out_acc = outp.tile([1, BC], f32)
        out_fin = outp.tile([1, BC], f32)

        rep_ctx = tc.For_i(0, repeat, 1) if repeat > 1 else None
        if rep_ctx is not None:
            ctx.enter_context(rep_ctx)

        def stage_a(x4_t, ch4g, cs):
            zA = zpsA.tile([128, 2 * CP_], f32, tag="zA")
            zB = zpsB.tile([128, 2 * CP_], f32, tag="zB")
            for j, (zt, off) in enumerate(
                    [(zA, 0), (zA, CP_), (zB, 0), (zB, CP_)]):
                nc.tensor.matmul(
                    zt[:, off:off + CP_],
                    linw_t[32 * j:32 * (j + 1), 128 * j:128 * (j + 1)],
                    x4_t[32 * j:32 * (j + 1), cs],
                    start=True, stop=True,
                    tile_position=(32 * j, 0),
                )
            chA = chp.tile([128, 2 * CP_], bf16, tag="chA")
            chB = chp.tile([128, 2 * CP_], bf16, tag="chB")
            nc.scalar.activation(chA[:], zA[:], SQ)
            nc.vector._custom_dve(SQUARE_ANT, out=chB[:], in0=zB[:])
            nc.gpsimd.tensor_mul(ch4g[0:32, cs], x4_t[0:32, cs], x4_t[0:32, cs])
            return (chA, chB, ch4g, cs)

        def stage_b(st, pr_t, pc):
            chA, chB, ch4g, cs = st
            yt = yps.tile([128, CP_], f32, tag="y")
            rhs = [chA[:, 0:CP_], chA[:, CP_:2 * CP_],
                   chB[:, 0:CP_], chB[:, CP_:2 * CP_]]
            for j in range(4):
                nc.tensor.matmul(yt[:], rwb03_t[:, 128 * j:128 * (j + 1)],
                                 rhs[j], start=(j == 0), stop=False)
            nc.tensor.matmul(yt[:], rwb4_t[:], ch4g[0:65, cs],
                             start=False, stop=True)
            h = CP_ // 2
            nc.scalar.activation(pr_t[:, pc:pc + h], yt[:, 0:h], SQ)
            nc.vector._custom_dve(SQUARE_ANT, out=pr_t[:, pc + h:pc + CP_],
                                  in0=yt[:, h:CP_])

        NQ = NCH // QCH
        for q in range(NQ):                    # tail quarters (64 batches)
            pr_t = prp.tile([128, QCH * CP_], bf16, tag="pr")
            pend = None
            for gg in range(QCH // GROUP):     # DMA groups within quarter
                g = q * (QCH // GROUP) + gg
                b0 = g * GROUP * CB
                nb = GROUP * CB
                xsrc = x_d[b0:b0 + nb].transpose([1, 0, 2])   # (32, nb, 64)
                x4_t = x4p.tile([128, GROUP * CP_], f32r, tag="x4")
                for j in range(4):
                    nc.sync.dma_start(
                        x4_t[32 * j:32 * (j + 1)].rearrange(
                            "k (b d) -> k b d", b=nb), xsrc)
                ch4g = ch4p.tile([65, GROUP * CP_], f32r, tag="ch4")
                nc.sync.dma_start(
                    ch4g[32:64].rearrange("k (b d) -> k b d", b=nb), xsrc)
                if g < 2:
                    nc.sync.dma_start(ch4g[64:65], ones_d[:])

                for ci in range(GROUP):
                    cs = slice(ci * CP_, (ci + 1) * CP_)
                    c_in_q = gg * GROUP + ci
                    st = stage_a(x4_t, ch4g, cs)
                    if pend is not None:
                        stage_b(*pend)
                    pend = (st, pr_t, c_in_q * CP_)
            stage_b(*pend)
            qacc = qps.tile([1, CP_], f32, tag="qacc")
            pr4 = pr_t[:].rearrange("p (b d) -> p b d", d=D)
            for dd in range(8):
                nc.tensor.matmul(
                    qacc[:], signs_t[:], pr4[:, :, 8 * dd:8 * (dd + 1)],
                    start=(dd == 0), stop=(dd == 7),
                )
            nc.vector.tensor_reduce(
                out_acc[0:1, q * QCH * CB:(q + 1) * QCH * CB],
                qacc[:].rearrange("p (b d) -> p b d", d=8),
                mybir.AxisListType.X, mybir.AluOpType.add,
            )
        nc.scalar.activation(out_fin[:], out_acc[:],
                             mybir.ActivationFunctionType.Copy,
                             bias=float(bconst))
        nc.sync.dma_start(out_d[:], out_fin[:])

    nc.compile()
    _module_cache[key] = nc
    return nc


def build_module5(bconst: float, repeat: int = 1):
    """v5: PSUM bank recycling for deep PE pipelining.

    - ringA [128,1024] f32 x2: chains 0,1 z + (after the square reads them)
      the yt accumulator reuses cols 0:512 of the same banks (WAR dep).
    - ringB/ringC [128,512] x2: chains 2,3; qacc tag-shares ringB.
    - squares: scalar = sqA (1024 cols) + pr (512); vector = sqB + sqC (512
      each) -> vector FIFO never waits on stage_b.
    """
    key = ("v5", round(bconst, 12), repeat)
    if key in _module_cache:
        return _module_cache[key]
    nc = bacc.Bacc("TRN2", target_bir_lowering=False)
    x_d = nc.dram_tensor("x", [BC, F, D], f32r, kind="ExternalInput")
    linw_d = nc.dram_tensor("linw2", [128, 4 * 128], f32r, kind="ExternalInput")
    rwb03_d = nc.dram_tensor("rwb03", [128, 4 * 128], bf16, kind="ExternalInput")
    rwb4_d = nc.dram_tensor("rwb4", [65, 128], f32r, kind="ExternalInput")
    signs_d = nc.dram_tensor("signs", [128, 1], bf16, kind="ExternalInput")
    ones_d = nc.dram_tensor("ones2", [1, 4 * 512], f32r, kind="ExternalInput")
    out_d = nc.dram_tensor("out", [1, BC], f32, kind="ExternalOutput")

    SQ = mybir.ActivationFunctionType.Square
    CP_ = 512
    CB = CP_ // D              # 8 batches per chunk
    NCH = BC // CB             # 32 chunks
    GROUP = 4
    QCH = 8

    with tile.TileContext(nc) as tc, ExitStack() as ctx:
        const = ctx.enter_context(tc.tile_pool(name="const", bufs=1))
        x4p = ctx.enter_context(tc.tile_pool(name="x4p", bufs=2))
        ch4p = ctx.enter_context(tc.tile_pool(name="ch4p", bufs=2))
        chp = ctx.enter_context(tc.tile_pool(name="chp", bufs=2))
        prp = ctx.enter_context(tc.tile_pool(name="prp", bufs=2))
        outp = ctx.enter_context(tc.tile_pool(name="outp", bufs=1))
        ringA = ctx.enter_context(tc.tile_pool(name="ringA", bufs=2, space="PSUM"))
        ringB = ctx.enter_context(tc.tile_pool(name="ringB", bufs=2, space="PSUM"))
        ringC = ctx.enter_context(tc.tile_pool(name="ringC", bufs=2, space="PSUM"))

        linw_t = const.tile([128, 4 * 128], f32r)
        nc.sync.dma_start(linw_t[:], linw_d[:])
        rwb03_t = const.tile([128, 4 * 128], bf16)
        nc.sync.dma_start(rwb03_t[:], rwb03_d[:])
        rwb4_t = const.tile([65, 128], f32r)
        nc.sync.dma_start(rwb4_t[:], rwb4_d[:])
        signs_t = const.tile([128, 1], bf16)
        nc.sync.dma_start(signs_t[:], signs_d[:])
        out_acc = outp.tile([1, BC], f32)
        out_fin = outp.tile([1, BC], f32)

        rep_ctx = tc.For_i(0, repeat, 1) if repeat > 1 else None
        if rep_ctx is not None:
            ctx.enter_context(rep_ctx)

        def stage_a(x4_t, ch4g, cs):
            zA = ringA.tile([128, 2 * CP_], f32, tag="zA")
            zB = ringB.tile([128, CP_], f32, tag="zB")
            zC = ringC.tile([128, CP_], f32, tag="zC")
            for j, (dst, off) in enumerate(
                    [(zA, 0), (zA, CP_), (zB, 0), (zC, 0)]):
                nc.tensor.matmul(
                    dst[:, off:off + CP_],
                    linw_t[32 * j:32 * (j + 1), 128 * j:128 * (j + 1)],
                    x4_t[32 * j:32 * (j + 1), cs],
                    start=True, stop=True,
                    tile_position=(32 * j, 0),
                )
            chA = chp.tile([128, 2 * CP_], bf16, tag="chA")
            chB = chp.tile([128, CP_], bf16, tag="chB")
            chC = chp.tile([128, CP_], bf16, tag="chC")
            nc.scalar.activation(chA[:], zA[:], SQ)
            nc.vector._custom_dve(SQUARE_ANT, out=chB[:], in0=zB[:])
            nc.vector._custom_dve(SQUARE_ANT, out=chC[:], in0=zC[:])
            nc.gpsimd.tensor_mul(ch4g[0:32, cs], x4_t[0:32, cs], x4_t[0:32, cs])
            return (chA, chB, chC, ch4g, cs, zA)

        def stage_b(st, pr_t, pc):
            chA, chB, chC, ch4g, cs, zA = st
            yt = zA[:, 0:CP_]          # recycle bank 0 of this chunk's zA
            rhs = [chA[:, 0:CP_], chA[:, CP_:2 * CP_], chB[:], chC[:]]
            for j in range(4):
                nc.tensor.matmul(yt, rwb03_t[:, 128 * j:128 * (j + 1)],
                                 rhs[j], start=(j == 0), stop=False)
            nc.tensor.matmul(yt, rwb4_t[:], ch4g[0:65, cs],
                             start=False, stop=True)
            nc.scalar.activation(pr_t[:, pc:pc + CP_], yt, SQ)

        NQ = NCH // QCH
        for q in range(NQ):
            pr_t = prp.tile([128, QCH * CP_], bf16, tag="pr")
            pend = None
            for gg in range(QCH // GROUP):
                g = q * (QCH // GROUP) + gg
                b0 = g * GROUP * CB
                nb = GROUP * CB
                xsrc = x_d[b0:b0 + nb].transpose([1, 0, 2])   # (32, nb, 64)
                x4_t = x4p.tile([128, GROUP * CP_], f32r, tag="x4")
                for j in range(4):
                    nc.sync.dma_start(
                        x4_t[32 * j:32 * (j + 1)].rearrange(
                            "k (b d) -> k b d", b=nb), xsrc)
                ch4g = ch4p.tile([65, GROUP * CP_], f32r, tag="ch4")
                nc.sync.dma_start(
                    ch4g[32:64].rearrange("k (b d) -> k b d", b=nb), xsrc)
                if g < 2:
                    nc.sync.dma_start(ch4g[64:65], ones_d[:])

                for ci in range(GROUP):
                    cs = slice(ci * CP_, (ci + 1) * CP_)
                    c_in_q = gg * GROUP + ci
                    st = stage_a(x4_t, ch4g, cs)
                    if pend is not None:
                        stage_b(*pend)
                    pend = (st, pr_t, c_in_q * CP_)
            stage_b(*pend)
            qacc = ringB.tile([1, CP_], f32, tag="zB")
            pr4 = pr_t[:].rearrange("p (b d) -> p b d", d=D)
            for dd in range(8):
                nc.tensor.matmul(
                    qacc[:], signs_t[:], pr4[:, :, 8 * dd:8 * (dd + 1)],
                    start=(dd == 0), stop=(dd == 7),
                )
            nc.vector.tensor_reduce(
                out_acc[0:1, q * QCH * CB:(q + 1) * QCH * CB],
                qacc[:].rearrange("p (b d) -> p b d", d=8),
                mybir.AxisListType.X, mybir.AluOpType.add,
            )
        nc.scalar.activation(out_fin[:], out_acc[:],
                             mybir.ActivationFunctionType.Copy,
                             bias=float(bconst))
        nc.sync.dma_start(out_d[:], out_fin[:])

    nc.compile()
    _module_cache[key] = nc
    return nc


def build_module6(bconst: float, repeat: int = 1, warmup: int = 14):
    """v6: v5 + PE warmup burst (trip HAM to 2.4GHz during the DMA lead-in)
    + quarter tails interleaved into the next quarter's chunk pipeline so
    scalar/vector queues never drain at quarter boundaries."""
    key = ("v6", round(bconst, 12), repeat, warmup)
    if key in _module_cache:
        return _module_cache[key]
    nc = bacc.Bacc("TRN2", target_bir_lowering=False)
    x_d = nc.dram_tensor("x", [BC, F, D], f32r, kind="ExternalInput")
    linw_d = nc.dram_tensor("linw2", [128, 4 * 128], f32r, kind="ExternalInput")
    rwb03_d = nc.dram_tensor("rwb03", [128, 4 * 128], bf16, kind="ExternalInput")
    rwb4_d = nc.dram_tensor("rwb4", [65, 128], f32r, kind="ExternalInput")
    signs_d = nc.dram_tensor("signs", [128, 1], bf16, kind="ExternalInput")
    ones_d = nc.dram_tensor("ones2", [1, 4 * 512], f32r, kind="ExternalInput")
    out_d = nc.dram_tensor("out", [1, BC], f32, kind="ExternalOutput")

    SQ = mybir.ActivationFunctionType.Square
    CP_ = 512
    CB = CP_ // D
    NCH = BC // CB             # 32 chunks
    GROUP = 4
    QCH = 8

    with tile.TileContext(nc) as tc, ExitStack() as ctx:
        const = ctx.enter_context(tc.tile_pool(name="const", bufs=1))
        x4p = ctx.enter_context(tc.tile_pool(name="x4p", bufs=2))
        ch4p = ctx.enter_context(tc.tile_pool(name="ch4p", bufs=2))
        chp = ctx.enter_context(tc.tile_pool(name="chp", bufs=2))
        prp = ctx.enter_context(tc.tile_pool(name="prp", bufs=2))
        outp = ctx.enter_context(tc.tile_pool(name="outp", bufs=1))
        ringA = ctx.enter_context(tc.tile_pool(name="ringA", bufs=2, space="PSUM"))
        ringB = ctx.enter_context(tc.tile_pool(name="ringB", bufs=2, space="PSUM"))
        ringC = ctx.enter_context(tc.tile_pool(name="ringC", bufs=2, space="PSUM"))

        linw_t = const.tile([128, 4 * 128], f32r)
        nc.sync.dma_start(linw_t[:], linw_d[:])
        rwb03_t = const.tile([128, 4 * 128], bf16)
        nc.sync.dma_start(rwb03_t[:], rwb03_d[:])
        rwb4_t = const.tile([65, 128], f32r)
        nc.sync.dma_start(rwb4_t[:], rwb4_d[:])
        signs_t = const.tile([128, 1], bf16)
        nc.sync.dma_start(signs_t[:], signs_d[:])
        out_acc = outp.tile([1, BC], f32)
        out_fin = outp.tile([1, BC], f32)

        rep_ctx = tc.For_i(0, repeat, 1) if repeat > 1 else None
        if rep_ctx is not None:
            ctx.enter_context(rep_ctx)

        # --- PE warmup: serial K=128 N=512 matmuls while group-0 DMA lands.
        wps = None
        if warmup:
            wps = ringA.tile([128, 2 * CP_], f32, tag="zA", name="warm")
            for w in range(warmup):
                nc.tensor.matmul(wps[:, 0:CP_], linw_t[:, 0:128],
                                 linw_t[:, 0:CP_], start=True, stop=True)

        def stage_a(x4_t, ch4g, cs, fill=0):
            zA = ringA.tile([128, 2 * CP_], f32, tag="zA")
            # filler matmuls keep HAM warm across dependency stalls; the real
            # chain-0 matmul below overwrites them (start=True clears bank).
            for w in range(fill):
                nc.tensor.matmul(zA[0:1, 0:CP_], signs_t[:],
                                 rwb03_t[:, 0:CP_], start=True, stop=True)
            zB = ringB.tile([128, CP_], f32, tag="zB")
            zC = ringC.tile([128, CP_], f32, tag="zC")
            for j, (dst, off) in enumerate(
                    [(zA, 0), (zA, CP_), (zB, 0), (zC, 0)]):
                nc.tensor.matmul(
                    dst[:, off:off + CP_],
                    linw_t[32 * j:32 * (j + 1), 128 * j:128 * (j + 1)],
                    x4_t[32 * j:32 * (j + 1), cs],
                    start=True, stop=True,
                    tile_position=(32 * j, 0),
                )
            chA = chp.tile([128, 2 * CP_], bf16, tag="chA")
            chB = chp.tile([128, CP_], bf16, tag="chB")
            chC = chp.tile([128, CP_], bf16, tag="chC")
            nc.scalar.activation(chA[:], zA[:], SQ)
            nc.vector._custom_dve(SQUARE_ANT, out=chB[:], in0=zB[:])
            nc.vector._custom_dve(SQUARE_ANT, out=chC[:], in0=zC[:])
            nc.gpsimd.tensor_mul(ch4g[0:32, cs], x4_t[0:32, cs], x4_t[0:32, cs])
            return (chA, chB, chC, ch4g, cs, zA)

        def stage_b(st, pr_t, pc):
            chA, chB, chC, ch4g, cs, zA = st
            yt = zA[:, 0:CP_]
            rhs = [chA[:, 0:CP_], chA[:, CP_:2 * CP_], chB[:], chC[:]]
            for j in range(4):
                nc.tensor.matmul(yt, rwb03_t[:, 128 * j:128 * (j + 1)],
                                 rhs[j], start=(j == 0), stop=False)
            nc.tensor.matmul(yt, rwb4_t[:], ch4g[0:65, cs],
                             start=False, stop=True)
            nc.scalar.activation(pr_t[:, pc:pc + CP_], yt, SQ)

        def tail(pr_t, q):
            qacc = ringB.tile([1, CP_], f32, tag="zB", name="qacc")
            pr4 = pr_t[:].rearrange("p (b d) -> p b d", d=D)
            for dd in range(8):
                nc.tensor.matmul(
                    qacc[:], signs_t[:], pr4[:, :, 8 * dd:8 * (dd + 1)],
                    start=(dd == 0), stop=(dd == 7),
                )
            nc.vector.tensor_reduce(
                out_acc[0:1, q * QCH * CB:(q + 1) * QCH * CB],
                qacc[:].rearrange("p (b d) -> p b d", d=8),
                mybir.AxisListType.X, mybir.AluOpType.add,
            )

        pend = None
        pend_tail = None           # (pr_t, q) awaiting tail issue
        pr_tiles = {}
        for c in range(NCH):
            q = c // QCH
            if c % QCH == 0:
                pr_tiles[q] = prp.tile([128, QCH * CP_], bf16, tag="pr",
                                       name=f"pr{q}")
            if c % GROUP == 0:
                g = c // GROUP
                b0 = g * GROUP * CB
                nb = GROUP * CB
                xsrc = x_d[b0:b0 + nb].transpose([1, 0, 2])   # (32, nb, 64)
                x4_t = x4p.tile([128, GROUP * CP_], f32r, tag="x4")
                for j in range(4):
                    nc.sync.dma_start(
                        x4_t[32 * j:32 * (j + 1)].rearrange(
                            "k (b d) -> k b d", b=nb), xsrc)
                ch4g = ch4p.tile([65, GROUP * CP_], f32r, tag="ch4")
                nc.sync.dma_start(
                    ch4g[32:64].rearrange("k (b d) -> k b d", b=nb), xsrc)
                if g < 2:
                    nc.sync.dma_start(ch4g[64:65], ones_d[:])

            cs = slice((c % GROUP) * CP_, (c % GROUP + 1) * CP_)
            st = stage_a(x4_t, ch4g, cs, fill=3 if c < 4 else 1)
            if pend is not None:
                stage_b(*pend)
            if pend_tail is not None and c >= pend_tail[0]:
                tail(*pend_tail[1:])
                pend_tail = None
            pend = (st, pr_tiles[q], (c % QCH) * CP_)
            if c % QCH == QCH - 1:
                pend_tail = (c + 2, pr_tiles[q], q)
        stage_b(*pend)
        if pend_tail is not None:
            tail(*pend_tail[1:])
        nc.scalar.activation(out_fin[:], out_acc[:],
                             mybir.ActivationFunctionType.Copy,
                             bias=float(bconst))
        nc.sync.dma_start(out_d[:], out_fin[:])

    nc.compile()
    _module_cache[key] = nc
    return nc


CFG7 = {"xbf": True, "warmup": 11}


def build_module7(bconst: float, repeat: int = 1):
    """v7: v2's proven-concurrent PSUM layout (zA/zB pairs bufs=1, yt bufs=2,
    qacc bufs=2) + bf16 LIN (exact 0/1 weights, FWL) + bf16 x for the z path
    (chain 4 stays f32r via the separate ch4g load) + warmup + interleaved
    tails."""
    xbf = CFG7["xbf"]
    warmup = CFG7["warmup"]
    key = ("v7", round(bconst, 12), repeat, xbf, warmup)
    if key in _module_cache:
        return _module_cache[key]
    nc = bacc.Bacc("TRN2", target_bir_lowering=False)
    xdt = bf16 if xbf else f32r
    x_d = nc.dram_tensor("x", [F, BC, D], f32r, kind="ExternalInput")
    xb_d = (nc.dram_tensor("xb", [F, BC, D], bf16, kind="ExternalInput")
            if xbf else x_d)
    linw_d = nc.dram_tensor("linw2", [128, 4 * 128], xdt, kind="ExternalInput")
    rwb03_d = nc.dram_tensor("rwb03", [128, 4 * 128], bf16, kind="ExternalInput")
    rwb4_d = nc.dram_tensor("rwb4", [65, 128], f32r, kind="ExternalInput")
    signs_d = nc.dram_tensor("signs", [128, 1], bf16, kind="ExternalInput")
    ones_d = nc.dram_tensor("ones2", [1, 24 * 512], f32r, kind="ExternalInput")
    out_d = nc.dram_tensor("out", [1, BC], f32, kind="ExternalOutput")

    SQ = mybir.ActivationFunctionType.Square
    CP_ = 512
    CB = CP_ // D
    NCH = BC // CB
    GROUP = 16
    QCH = 8

    with tile.TileContext(nc) as tc, ExitStack() as ctx:
        const = ctx.enter_context(tc.tile_pool(name="const", bufs=1))
        x4p = ctx.enter_context(tc.tile_pool(name="x4p", bufs=2))
        ch4p = ctx.enter_context(tc.tile_pool(name="ch4p", bufs=2))
        chp = ctx.enter_context(tc.tile_pool(name="chp", bufs=2))
        prp = ctx.enter_context(tc.tile_pool(name="prp", bufs=2))
        outp = ctx.enter_context(tc.tile_pool(name="outp", bufs=1))
        zpsA = ctx.enter_context(tc.tile_pool(name="zpsA", bufs=1, space="PSUM"))
        zpsB = ctx.enter_context(tc.tile_pool(name="zpsB", bufs=2, space="PSUM"))
        yps = ctx.enter_context(tc.tile_pool(name="yps", bufs=2, space="PSUM"))

        linw_t = const.tile([128, 4 * 128], xdt)
        nc.sync.dma_start(linw_t[:], linw_d[:])
        rwb03_t = const.tile([128, 4 * 128], bf16)
        nc.sync.dma_start(rwb03_t[:], rwb03_d[:])
        rwb4_t = const.tile([65, 128], f32r)
        nc.sync.dma_start(rwb4_t[:], rwb4_d[:])
        signs_t = const.tile([128, 1], bf16)
        nc.sync.dma_start(signs_t[:], signs_d[:])
        out_acc = outp.tile([1, BC], f32)
        out_fin = outp.tile([1, BC], f32)

        rep_ctx = tc.For_i(0, repeat, 1) if repeat > 1 else None
        if rep_ctx is not None:
            ctx.enter_context(rep_ctx)

        if warmup:
            wq = yps.tile([128, CP_], f32, tag="y", name="warm")
            for w in range(warmup):
                nc.tensor.matmul(wq[:], rwb03_t[:, 0:128], rwb03_t[:, 0:CP_],
                                 start=True, stop=True)

        def stage_a(x4_t, ch4g, cs):
            zA = zpsA.tile([128, 2 * CP_], f32, tag="zA")
            zB = zpsB.tile([128, 2 * CP_], f32, tag="zB")
            for j, (zt, off) in enumerate(
                    [(zA, 0), (zA, CP_), (zB, 0), (zB, CP_)]):
                nc.tensor.matmul(
                    zt[:, off:off + CP_],
                    linw_t[32 * j:32 * (j + 1), 128 * j:128 * (j + 1)],
                    x4_t[32 * j:32 * (j + 1), cs],
                    start=True, stop=True,
                    tile_position=(32 * j, 0),
                )
            chA = chp.tile([128, 2 * CP_], bf16, tag="chA")
            chB = chp.tile([128, 2 * CP_], bf16, tag="chB")
            nc.scalar.activation(chA[:], zA[:], SQ)
            nc.vector._custom_dve(SQUARE_ANT, out=chB[:], in0=zB[:])
            nc.gpsimd.tensor_mul(ch4g[0:32, cs], ch4g[32:64, cs],
                                 ch4g[32:64, cs])
            return (chA, chB, ch4g, cs)

        def stage_b1(st):
            chA, chB, ch4g, cs = st
            yt = yps.tile([128, CP_], f32, tag="y")
            rhs = [chA[:, 0:CP_], chA[:, CP_:2 * CP_],
                   chB[:, 0:CP_], chB[:, CP_:2 * CP_]]
            for j in range(4):
                nc.tensor.matmul(yt[:], rwb03_t[:, 128 * j:128 * (j + 1)],
                                 rhs[j], start=(j == 0), stop=False)
            nc.tensor.matmul(yt[:], rwb4_t[:], ch4g[0:65, cs],
                             start=False, stop=True)
            return yt

        def stage_b2(yt, pr_t, pc):
            nc.scalar.activation(pr_t[:, pc:pc + CP_], yt[:], SQ)

        def tail(pr_t, q):
            qacc = yps.tile([1, CP_], f32, tag="y")
            pr4 = pr_t[:].rearrange("p (b d) -> p b d", d=D)
            for dd in range(8):
                nc.tensor.matmul(
                    qacc[:], signs_t[:], pr4[:, :, 8 * dd:8 * (dd + 1)],
                    start=(dd == 0), stop=(dd == 7),
                )
            return qacc

        def tail_red(qacc, q):
            nc.vector.tensor_reduce(
                out_acc[0:1, q * QCH * CB:(q + 1) * QCH * CB],
                qacc[:].rearrange("p (b d) -> p b d", d=8),
                mybir.AxisListType.X, mybir.AluOpType.add,
            )

        pend = None            # stage_a result awaiting big matmuls (dist 1)
        pend2 = None           # yt awaiting pr square (dist 2)
        pend_tail = None
        pend_red = None
        pr_tiles = {}
        for c in range(NCH):
            q = c // QCH
            if c % QCH == 0:
                pr_tiles[q] = prp.tile([128, QCH * CP_], bf16, tag="pr",
                                       name=f"pr{q}")
            if c in (0, 8):            # asymmetric groups: 8 then 24 chunks
                gsz = 8 if c == 0 else 24
                gstart = c
                b0 = c * CB
                nb = gsz * CB
                nsl = 2 if c == 0 else 2   # slices per group
                nh = nb // nsl
                x4_t = x4p.tile([128, gsz * CP_], xdt, tag="x4")
                ch4g = ch4p.tile([65, gsz * CP_], f32r, tag="ch4")
                for h in range(nsl):
                    hb = b0 + h * nh
                    hc = slice(h * nh * D, (h + 1) * nh * D)
                    xsrc = xb_d[:, hb:hb + nh, :]
                    for j in range(4):
                        nc.sync.dma_start(
                            x4_t[32 * j:32 * (j + 1), hc].rearrange(
                                "k (b d) -> k b d", b=nh), xsrc)
                    nc.sync.dma_start(
                        ch4g[32:64, hc].rearrange("k (b d) -> k b d", b=nh),
                        x_d[:, hb:hb + nh, :])
                nc.sync.dma_start(ch4g[64:65], ones_d[0:1, 0:gsz * 512])

            cs = slice((c - gstart) * CP_, (c - gstart + 1) * CP_)
            if pend2 is not None:
                stage_b2(*pend2)
                pend2 = None
            st = stage_a(x4_t, ch4g, cs)
            if pend is not None:
                yt = stage_b1(pend[0])
                pend2 = (yt, pend[1], pend[2])
            if pend_tail is not None and c >= pend_tail[0]:
                tail_red(tail(*pend_tail[1:]), pend_tail[2])
                pend_tail = None
            pend = (st, pr_tiles[q], (c % QCH) * CP_)
            if c % QCH == QCH - 1:
                pend_tail = (c + 3, pr_tiles[q], q)
        yt = stage_b1(pend[0])
        stage_b2(*pend2)
        stage_b2(yt, pend[1], pend[2])
        if pend_tail is not None:
            tail_red(tail(*pend_tail[1:]), pend_tail[2])
        nc.scalar.activation(out_fin[:], out_acc[:],
                             mybir.ActivationFunctionType.Copy,
                             bias=float(bconst))
        nc.sync.dma_start(out_d[:], out_fin[:])

    nc.compile()
    _module_cache[key] = nc
    return nc


def _run7(inputs, trace=False, **kw):
    folded, bconst, _info = fold_weights2(
        inputs["W1"], inputs["b1"], inputs["W2"], inputs["b2"],
        inputs["W3"], inputs["b3"], inputs["W_out"], inputs["b_out"],
    )
    import ml_dtypes
    nc = build_module7(bconst)
    x0 = np.ascontiguousarray(np.asarray(inputs["x0"], dtype=np.float32))
    rwb = folded["rwb"]
    in_maps = []
    for c in range(NCORES):
        xs = np.ascontiguousarray(x0[BC * c:BC * (c + 1)])
        xt = np.ascontiguousarray(xs.transpose(1, 0, 2))   # (F, BC, D)
        m = {
            "linw2": folded["linw2"].astype(
                ml_dtypes.bfloat16 if CFG7["xbf"] else np.float32),
            "rwb03": np.ascontiguousarray(rwb[:, 0:512]).astype(ml_dtypes.bfloat16),
            "rwb4": np.ascontiguousarray(rwb[0:65, 512:640]),
            "signs": folded["signs"].astype(ml_dtypes.bfloat16),
            "ones2": np.ones((1, 24 * 512), dtype=np.float32),
            "x": xt,
        }
        if CFG7["xbf"]:
            m["xb"] = xt.astype(ml_dtypes.bfloat16)
        in_maps.append(m)
    res = run_bass_kernel_spmd(nc, in_maps, core_ids=list(range(NCORES)),
                               trace=trace, **kw)
    out = np.concatenate(
        [res.results[c]["out"].reshape(BC, 1) for c in range(NCORES)], axis=0
    )
    return out, res


def _run6(inputs, trace=False, **kw):
    folded, bconst, _info = fold_weights2(
        inputs["W1"], inputs["b1"], inputs["W2"], inputs["b2"],
        inputs["W3"], inputs["b3"], inputs["W_out"], inputs["b_out"],
    )
    import ml_dtypes
    nc = build_module6(bconst)
    x0 = np.ascontiguousarray(np.asarray(inputs["x0"], dtype=np.float32))
    rwb = folded["rwb"]
    in_maps = []
    for c in range(NCORES):
        m = {
            "linw2": folded["linw2"],
            "rwb03": np.ascontiguousarray(rwb[:, 0:512]).astype(ml_dtypes.bfloat16),
            "rwb4": np.ascontiguousarray(rwb[0:65, 512:640]),
            "signs": folded["signs"].astype(ml_dtypes.bfloat16),
            "ones2": np.ones((1, 4 * 512), dtype=np.float32),
            "x": np.ascontiguousarray(x0[BC * c:BC * (c + 1)]),
        }
        in_maps.append(m)
    res = run_bass_kernel_spmd(nc, in_maps, core_ids=list(range(NCORES)),
                               trace=trace, **kw)
    out = np.concatenate(
        [res.results[c]["out"].reshape(BC, 1) for c in range(NCORES)], axis=0
    )
    return out, res


def _run5(inputs, trace=False, **kw):
    folded, bconst, _info = fold_weights2(
        inputs["W1"], inputs["b1"], inputs["W2"], inputs["b2"],
        inputs["W3"], inputs["b3"], inputs["W_out"], inputs["b_out"],
    )
    import ml_dtypes
    nc = build_module5(bconst)
    x0 = np.ascontiguousarray(np.asarray(inputs["x0"], dtype=np.float32))
    rwb = folded["rwb"]
    in_maps = []
    for c in range(NCORES):
        m = {
            "linw2": folded["linw2"],
            "rwb03": np.ascontiguousarray(rwb[:, 0:512]).astype(ml_dtypes.bfloat16),
            "rwb4": np.ascontiguousarray(rwb[0:65, 512:640]),
            "signs": folded["signs"].astype(ml_dtypes.bfloat16),
            "ones2": np.ones((1, 4 * 512), dtype=np.float32),
            "x": np.ascontiguousarray(x0[BC * c:BC * (c + 1)]),
        }
        in_maps.append(m)
    res = run_bass_kernel_spmd(nc, in_maps, core_ids=list(range(NCORES)),
                               trace=trace, **kw)
    out = np.concatenate(
        [res.results[c]["out"].reshape(BC, 1) for c in range(NCORES)], axis=0
    )
    return out, res


def _run4(inputs, trace=False, **kw):
    folded, bconst, _info = fold_weights2(
        inputs["W1"], inputs["b1"], inputs["W2"], inputs["b2"],
        inputs["W3"], inputs["b3"], inputs["W_out"], inputs["b_out"],
    )
    import ml_dtypes
    nc = build_module4(bconst)
    x0 = np.ascontiguousarray(np.asarray(inputs["x0"], dtype=np.float32))
    rwb = folded["rwb"]
    in_maps = []
    for c in range(NCORES):
        m = {
            "linw2": folded["linw2"],
            "rwb03": np.ascontiguousarray(rwb[:, 0:512]).astype(ml_dtypes.bfloat16),
            "rwb4": np.ascontiguousarray(rwb[0:65, 512:640]),
            "signs": folded["signs"].astype(ml_dtypes.bfloat16),
            "ones2": np.ones((1, 4 * 512), dtype=np.float32),
            "x": np.ascontiguousarray(x0[BC * c:BC * (c + 1)]),
        }
        in_maps.append(m)
    res = run_bass_kernel_spmd(nc, in_maps, core_ids=list(range(NCORES)),
                               trace=trace, **kw)
    out = np.concatenate(
        [res.results[c]["out"].reshape(BC, 1) for c in range(NCORES)], axis=0
    )
    return out, res


def _run3(inputs, trace=False, **kw):
    folded, bconst, _info = fold_weights2(
        inputs["W1"], inputs["b1"], inputs["W2"], inputs["b2"],
        inputs["W3"], inputs["b3"], inputs["W_out"], inputs["b_out"],
    )
    import ml_dtypes
    nc = build_module3(bconst)
    x0 = np.ascontiguousarray(np.asarray(inputs["x0"], dtype=np.float32))
    in_maps = []
    for c in range(NCORES):
        m = {
            "linw2": folded["linw2"],
            "rwb": folded["rwb"],
            "signs": folded["signs"].astype(ml_dtypes.bfloat16),
            "ones2": np.ones((1, 4 * 512), dtype=np.float32),
            "x": np.ascontiguousarray(x0[BC * c:BC * (c + 1)]),
        }
        in_maps.append(m)
    res = run_bass_kernel_spmd(nc, in_maps, core_ids=list(range(NCORES)),
                               trace=trace, **kw)
    out = np.concatenate(
        [res.results[c]["out"].reshape(BC, 1) for c in range(NCORES)], axis=0
    )
    return out, res


def _run2(inputs, trace=False, **kw):
    folded, bconst, _info = fold_weights2(
        inputs["W1"], inputs["b1"], inputs["W2"], inputs["b2"],
        inputs["W3"], inputs["b3"], inputs["W_out"], inputs["b_out"],
    )
    import ml_dtypes
    nc = build_module2(bconst)
    x0 = np.ascontiguousarray(np.asarray(inputs["x0"], dtype=np.float32))
    in_maps = []
    for c in range(NCORES):
        m = {
            "linw2": folded["linw2"],
            "rwb": folded["rwb"],
            "signs": folded["signs"].astype(ml_dtypes.bfloat16),
            "ones2": np.ones((1, CFG2["group"] * 512), dtype=np.float32),
            "x": np.ascontiguousarray(x0[BC * c:BC * (c + 1)]),
        }
        in_maps.append(m)
    res = run_bass_kernel_spmd(nc, in_maps, core_ids=list(range(NCORES)),
                               trace=trace, **kw)
    out = np.concatenate(
        [res.results[c]["out"].reshape(BC, 1) for c in range(NCORES)], axis=0
    )
    return out, res


def _run(inputs, trace=False, **kw):
    folded, bconst = fold_weights(
        inputs["W1"], inputs["b1"], inputs["W2"], inputs["b2"],
        inputs["W3"], inputs["b3"], inputs["W_out"], inputs["b_out"],
    )
    nc = build_module(bconst)
    x0 = np.ascontiguousarray(np.asarray(inputs["x0"], dtype=np.float32))
    in_maps = []
    for c in range(NCORES):
        m = dict(folded)
        m["x"] = np.ascontiguousarray(x0[BC * c:BC * (c + 1)])
        in_maps.append(m)
    res = run_bass_kernel_spmd(nc, in_maps, core_ids=list(range(NCORES)),
                               trace=trace, **kw)
    out = np.concatenate(
        [res.results[c]["out"].reshape(BC, 1) for c in range(NCORES)], axis=0
    )
    return out, res


def kernel(**inputs) -> np.ndarray:
    out, _ = _run7(inputs, trace=False)
    return out

